# revision 1
# baseline (speedup 1.0000x reference)
"""Causal self-attention (GQA + RMS-norm + RoPE) Trainium2 Bass kernel.

Sharding: 8 cores = 4 batches x 2 head-groups (tensor-parallel over heads).
Core c = 2*b + t handles batch b with Q heads [8t, 8t+8) and KV heads
[2t, 2t+2). Each core computes a partial output projection (its heads'
rows of W_out); the host sums the two partials per batch.

All matmuls run as float32r (full fp32 data, full-rate PE mode).

Pipeline per core:
  P1: qkv = x @ W_shard (transposed-x input), RMS+RoPE on q/k in natural
      layout, PE-transpose q/k to [d, tok], spill qT/kT/v to DRAM scratch.
  P2: per 512-token query window, per head: scoresT = kT_tile.T @ qT_win,
      +tri-mask on diagonal tiles, exp (ACT, scale=hd^-0.5), then
      yT += v_tile.T @ expT and sums += ones.T @ expT; normalize yT by
      broadcasted 1/sums.
  P3: out = sum_h yT_norm_h.T @ W_out_h rows -> partial [S, D].
"""
import sys, os
sys.path.insert(0, '/opt/trn_rl_repo')
import numpy as np

from concourse import bass, bacc, mybir, tile

f32 = mybir.dt.float32
f32r = mybir.dt.float32r

B, S, D = 4, 2048, 2048
H, HKV, HD = 16, 4, 128
HLOC = H // 2          # 8 q heads per core
KVLOC = HKV // 2       # 2 kv heads per core
SCALE = float(HD) ** -0.5
RMS_EPS = float(np.finfo(np.float32).eps)
ROPE_BASE = 10000.0

NTC = S // 128         # 16 token tiles
NDT = D // 128         # 16 contraction tiles
NWIN = S // 512        # 4 query windows


def _rope_tables():
    inv_freq = (1.0 / (ROPE_BASE ** (np.arange(0, HD, 2, dtype=np.float32) / HD))).astype(np.float32)
    freqs = np.arange(S, dtype=np.float32)[:, None] * inv_freq[None, :]
    cos = np.cos(freqs).astype(np.float32)
    sin = np.sin(freqs).astype(np.float32)
    cos2 = np.concatenate([cos, cos], axis=1)        # [S, 128]
    sin2 = np.concatenate([sin, -sin], axis=1)       # [S, 128]
    return cos2, sin2


def _tri_masks():
    # mask[vi][p, f] = -1e30 where kv > q for scoresT diag tiles:
    # kv = 128*j + p, q = 512*w + f, vi = j - 4*w -> masked iff p + 128*vi > f
    m = np.zeros((4, 128, 512), dtype=np.float32)
    p = np.arange(128)[:, None]
    f = np.arange(512)[None, :]
    for vi in range(4):
        m[vi][(p + 128 * vi) > f] = -1e30
    return m


def _emit_rms_rope(nc, scr, psum_ap, nheads, cos1, sin1, nat_tile, eps_ap):
    """psum_ap: [128, nheads*128] qkv psum slice; writes RMS+RoPE result into
    nat_tile (SBUF). cos1/sin1: [128, 1, 128] APs (cos duplicated, [sin,-sin]).

    rot(q) = q*cos2 + swap_halves(q)*sin2;  out = rot(q) * rsqrt(mean(q^2)+eps)
    rsqrt computed as exp(-0.5*ln(ss/128+eps)) on ACT (DVE reciprocal is slow).
    """
    w = nheads * 128
    sq = scr.tile([128, w], f32, tag="sq")
    nc.scalar.activation(sq[:], psum_ap, mybir.ActivationFunctionType.Square)
    ss = scr.tile([128, nheads, 1], f32, tag="ss")
    nc.vector.tensor_reduce(
        ss[:], sq[:].rearrange("p (h f) -> p h f", h=nheads),
        axis=mybir.AxisListType.X, op=mybir.AluOpType.add)
    lg = scr.tile([128, nheads, 1], f32, tag="lg")
    nc.scalar.activation(lg[:], ss[:], mybir.ActivationFunctionType.Ln,
                         scale=1.0 / HD, bias=eps_ap)
    rinv = scr.tile([128, nheads, 1, 1], f32, tag="rinv")
    nc.scalar.activation(rinv[:], lg[:], mybir.ActivationFunctionType.Exp,
                         scale=-0.5)

    shp = [128, nheads, 2, 64]
    p4 = psum_ap.rearrange("p (h x f) -> p h x f", h=nheads, x=2)
    p4s = p4[:, :, ::-1, :]
    cb = cos1.rearrange("p t (x f) -> p t x f", x=2).to_broadcast(shp)
    sb_ = sin1.rearrange("p t (x f) -> p t x f", x=2).to_broadcast(shp)
    rb = rinv[:].to_broadcast(shp)
    t1 = scr.tile(shp, f32, tag="t1")
    t2 = scr.tile(shp, f32, tag="t2")
    nc.vector.tensor_mul(t1[:], p4, cb)
    nc.vector.tensor_mul(t2[:], p4s, sb_)
    nc.vector.tensor_add(t1[:], t1[:], t2[:])
    nc.vector.tensor_mul(nat_tile[:].rearrange("p (h x f) -> p h x f", h=nheads, x=2),
                         t1[:], rb)


def build_program():
    cos_np, sin_np = _rope_tables()
    masks_np = _tri_masks()

    nc = bacc.Bacc(trn_type="TRN2")

    xt_d = nc.dram_tensor("xt", [D, S], f32, kind="ExternalInput")
    wq_d = nc.dram_tensor("wq", [D, HLOC * HD], f32, kind="ExternalInput")
    wkv_d = nc.dram_tensor("wkv", [D, 2 * KVLOC * HD], f32, kind="ExternalInput")
    wo_d = nc.dram_tensor("wo", [HLOC * HD, D], f32, kind="ExternalInput")
    out_d = nc.dram_tensor("out", [S, D], f32, kind="ExternalOutput")

    cos_d = nc.inline_tensor(cos_np, "cos_t")
    sin_d = nc.inline_tensor(sin_np, "sin_t")
    ident_d = nc.inline_tensor(np.eye(128, dtype=np.float32), "ident")
    masks_d = nc.inline_tensor(masks_np, "tri_masks")
    onescol_d = nc.inline_tensor(np.ones((128, 1), dtype=np.float32), "onescol")
    onesrow_d = nc.inline_tensor(np.ones((1, 128), dtype=np.float32), "onesrow")

    qt_scr = nc.dram_tensor("qt_scr", [HLOC, 128, S], f32)
    kt_scr = nc.dram_tensor("kt_scr", [KVLOC, 128, S], f32)
    v_scr = nc.dram_tensor("v_scr", [S, KVLOC * HD], f32)

    with tile.TileContext(nc) as tc:
        with tc.tile_pool(name="cst", bufs=1) as cst:
            cos_sb = cst.tile([128, NTC, 128], f32, tag="cos")
            sin_sb = cst.tile([128, NTC, 128], f32, tag="sin")
            ident = cst.tile([128, 128], f32, tag="ident")
            masks = cst.tile([128, 4, 512], f32, tag="masks")
            ones = cst.tile([128, 1], f32r, tag="ones")
            ones_r = cst.tile([1, 128], f32r, tag="ones_r")
            eps_sb = cst.tile([128, 1], f32, tag="eps")
            nc.sync.dma_start(out=cos_sb[:], in_=cos_d[:].rearrange("(t p) f -> p t f", p=128))
            nc.sync.dma_start(out=sin_sb[:], in_=sin_d[:].rearrange("(t p) f -> p t f", p=128))
            nc.sync.dma_start(out=ident[:], in_=ident_d[:])
            nc.sync.dma_start(out=masks[:], in_=masks_d[:].rearrange("v p f -> p v f"))
            nc.sync.dma_start(out=ones[:], in_=onescol_d[:].bitcast(f32r))
            nc.sync.dma_start(out=ones_r[:], in_=onesrow_d[:].bitcast(f32r))
            nc.gpsimd.memset(eps_sb[:], RMS_EPS)

            # ---------------- Phase 1: QKV projection ----------------
            with tc.tile_pool(name="w1", bufs=1) as w1, \
                 tc.tile_pool(name="xs", bufs=3) as xs, \
                 tc.tile_pool(name="nat", bufs=3) as nat, \
                 tc.tile_pool(name="stg", bufs=4) as stg, \
                 tc.tile_pool(name="p1a", bufs=6, space="PSUM") as p1a, \
                 tc.tile_pool(name="p1t", bufs=2, space="PSUM") as p1t:

                wq_sb = w1.tile([128, NDT, HLOC * HD], f32r, tag="wq")
                wkv_sb = w1.tile([128, NDT, 512], f32r, tag="wkv")
                wq_r = wq_d[:].bitcast(f32r).rearrange("(t p) c -> p t c", p=128)
                wkv_r = wkv_d[:].bitcast(f32r).rearrange("(t p) c -> p t c", p=128)
                for dt in range(NDT):
                    nc.sync.dma_start(out=wkv_sb[:, dt, :], in_=wkv_r[:, dt, :])
                for dt in range(NDT):
                    nc.scalar.dma_start(out=wq_sb[:, dt, :], in_=wq_r[:, dt, :])

                for tcid in range(NTC):
                    xt_sb = xs.tile([128, NDT, 128], f32r, tag="xt")
                    nc.sync.dma_start(
                        out=xt_sb[:],
                        in_=xt_d[:, tcid * 128:(tcid + 1) * 128]
                            .bitcast(f32r).rearrange("(t p) s -> p t s", p=128))

                    ps_q1 = p1a.tile([128, 512], f32, tag="acc")
                    ps_q2 = p1a.tile([128, 512], f32, tag="acc")
                    ps_kv = p1a.tile([128, 512], f32, tag="acc")
                    for dt in range(NDT):
                        st, sp = dt == 0, dt == NDT - 1
                        nc.tensor.matmul(ps_kv[:], xt_sb[:, dt, :], wkv_sb[:, dt, :], start=st, stop=sp)
                    for dt in range(NDT):
                        st, sp = dt == 0, dt == NDT - 1
                        lhs = xt_sb[:, dt, :]
                        nc.tensor.matmul(ps_q1[:], lhs, wq_sb[:, dt, 0:512], start=st, stop=sp)
                        nc.tensor.matmul(ps_q2[:], lhs, wq_sb[:, dt, 512:1024], start=st, stop=sp)

                    cos1 = cos_sb[:, tcid:tcid + 1, :]
                    sin1 = sin_sb[:, tcid:tcid + 1, :]

                    # q heads 0-3 / 4-7: RMS+RoPE, then PE-transpose to qT
                    for gi, ps in ((0, ps_q1), (1, ps_q2)):
                        qn = nat.tile([128, 512], f32, tag="qn")
                        _emit_rms_rope(nc, nat, ps[:], 4, cos1, sin1, qn, eps_sb[:])
                        for hh in range(4):
                            h = gi * 4 + hh
                            tp = p1t.tile([128, 128], f32, tag="tp")
                            nc.tensor.transpose(tp[:], qn[:, hh * 128:(hh + 1) * 128], ident[:])
                            sg = stg.tile([128, 128], f32, tag="sg")
                            nc.vector.tensor_copy(sg[:], tp[:])
                            nc.scalar.dma_start(
                                out=qt_scr[h][:, tcid * 128:(tcid + 1) * 128], in_=sg[:])

                    # k heads (cols 0:256 of kv psum)
                    kn = nat.tile([128, 256], f32, tag="kn")
                    _emit_rms_rope(nc, nat, ps_kv[:, 0:256], 2, cos1, sin1, kn, eps_sb[:])
                    for kh in range(KVLOC):
                        tp = p1t.tile([128, 128], f32, tag="tp")
                        nc.tensor.transpose(tp[:], kn[:, kh * 128:(kh + 1) * 128], ident[:])
                        sg = stg.tile([128, 128], f32, tag="sg")
                        nc.vector.tensor_copy(sg[:], tp[:])
                        nc.scalar.dma_start(
                            out=kt_scr[kh][:, tcid * 128:(tcid + 1) * 128], in_=sg[:])

                    # v: plain copy out (natural layout)
                    vn = nat.tile([128, 256], f32, tag="vn")
                    nc.vector.tensor_copy(vn[:], ps_kv[:, 256:512])
                    nc.scalar.dma_start(
                        out=v_scr[tcid * 128:(tcid + 1) * 128, :], in_=vn[:])

            # ------------- Phases 2+3 (share the ytn resident) -------------
            with tc.tile_pool(name="ytp", bufs=1) as ytp:
                ytn = ytp.tile([128, HLOC, S], f32r, tag="ytn")

                # ---- Phase 2: attention ----
                with tc.tile_pool(name="kv2", bufs=1) as kv2, \
                     tc.tile_pool(name="qw", bufs=2) as qw, \
                     tc.tile_pool(name="ex", bufs=6) as ex, \
                     tc.tile_pool(name="sm", bufs=4) as sm, \
                     tc.tile_pool(name="p2s", bufs=3, space="PSUM") as p2s, \
                     tc.tile_pool(name="p2y", bufs=3, space="PSUM") as p2y, \
                     tc.tile_pool(name="p2n", bufs=2, space="PSUM") as p2n:

                    kt_sb = kv2.tile([128, KVLOC, S], f32r, tag="kt")
                    v_sb = kv2.tile([128, NTC, KVLOC * HD], f32r, tag="v")
                    nc.sync.dma_start(out=kt_sb[:], in_=kt_scr[:].bitcast(f32r).rearrange("k p t -> p k t"))
                    nc.sync.dma_start(out=v_sb[:], in_=v_scr[:].bitcast(f32r).rearrange("(t p) c -> p t c", p=128))

                    for w in range(NWIN):
                        qt_win = qw.tile([128, HLOC, 512], f32r, tag="qtw")
                        nc.sync.dma_start(
                            out=qt_win[:],
                            in_=qt_scr[:, :, w * 512:(w + 1) * 512].bitcast(f32r).rearrange("h p t -> p h t"))
                        njt = 4 * w + 4
                        for hq in range(HLOC):
                            kvh = hq // 4
                            ps_y = p2y.tile([128, 512], f32, tag="y")
                            ps_s = p2n.tile([1, 512], f32, tag="s")
                            rhs_q = qt_win[:, hq, :]
                            for j in range(njt):
                                ps_sc = p2s.tile([128, 512], f32, tag="sc")
                                nc.tensor.matmul(
                                    ps_sc[:],
                                    kt_sb[:, kvh, j * 128:(j + 1) * 128],
                                    rhs_q)
                                if j >= 4 * w:
                                    nc.vector.tensor_add(ps_sc[:], ps_sc[:], masks[:, j - 4 * w, :])
                                et = ex.tile([128, 512], f32r, tag="et")
                                nc.scalar.activation(et[:], ps_sc[:],
                                                     mybir.ActivationFunctionType.Exp,
                                                     scale=SCALE)
                                st, sp = j == 0, j == njt - 1
                                nc.tensor.matmul(
                                    ps_y[:],
                                    v_sb[:, j, kvh * 128:(kvh + 1) * 128],
                                    et[:], start=st, stop=sp,
                                    skip_group_check=True)
                                nc.tensor.matmul(
                                    ps_s[:], ones[:], et[:],
                                    start=st, stop=sp, skip_group_check=True)
                            lgs = sm.tile([1, 512], f32, tag="lgs")
                            nc.scalar.activation(lgs[:], ps_s[:],
                                                 mybir.ActivationFunctionType.Ln)
                            rec = sm.tile([1, 512], f32r, tag="rec")
                            nc.scalar.activation(rec[:], lgs[:],
                                                 mybir.ActivationFunctionType.Exp,
                                                 scale=-1.0)
                            bcp = p2s.tile([128, 512], f32, tag="sc")
                            nc.tensor.matmul(bcp[:], ones_r[:], rec[:])
                            bc = sm.tile([128, 512], f32, tag="bc")
                            nc.vector.tensor_copy(bc[:], bcp[:])
                            nc.vector.tensor_mul(
                                ytn[:, hq, w * 512:(w + 1) * 512], ps_y[:], bc[:])

                # ---- Phase 3: output projection ----
                with tc.tile_pool(name="w3", bufs=1) as w3, \
                     tc.tile_pool(name="ob", bufs=4) as ob, \
                     tc.tile_pool(name="p3", bufs=4, space="PSUM") as p3:
                    wo_sb = w3.tile([128, HLOC, D], f32r, tag="wo")
                    wo_r = wo_d[:].bitcast(f32r).rearrange("(h p) c -> p h c", p=128)
                    for og in range(4):
                        nc.sync.dma_start(out=wo_sb[:, :, og * 512:(og + 1) * 512],
                                          in_=wo_r[:, :, og * 512:(og + 1) * 512])
                    for og in range(4):
                        for tcid in range(NTC):
                            ps_o = p3.tile([128, 512], f32, tag="o")
                            for h in range(HLOC):
                                nc.tensor.matmul(
                                    ps_o[:],
                                    ytn[:, h, tcid * 128:(tcid + 1) * 128],
                                    wo_sb[:, h, og * 512:(og + 1) * 512],
                                    start=(h == 0), stop=(h == HLOC - 1))
                            ot = ob.tile([128, 512], f32, tag="ot")
                            nc.vector.tensor_copy(ot[:], ps_o[:])
                            nc.scalar.dma_start(
                                out=out_d[tcid * 128:(tcid + 1) * 128, og * 512:(og + 1) * 512],
                                in_=ot[:])

    nc.compile()
    return nc


_PROGRAM = None


def _get_program():
    global _PROGRAM
    if _PROGRAM is None:
        _PROGRAM = build_program()
    return _PROGRAM


def make_in_maps(x, W_qkv, W_out):
    in_maps = []
    for c in range(8):
        b, t = c // 2, c % 2
        xt = np.ascontiguousarray(x[b].T)
        wq = np.ascontiguousarray(W_qkv[:, t * 1024:(t + 1) * 1024])
        wk = W_qkv[:, D + t * 256: D + (t + 1) * 256]
        wv = W_qkv[:, D + 512 + t * 256: D + 512 + (t + 1) * 256]
        wkv = np.ascontiguousarray(np.concatenate([wk, wv], axis=1))
        wo = np.ascontiguousarray(W_out[t * 1024:(t + 1) * 1024, :])
        in_maps.append({"xt": xt, "wq": wq, "wkv": wkv, "wo": wo})
    return in_maps


def kernel(x, W_qkv, W_out):
    from concourse.bass_utils import run_bass_kernel_spmd
    nc = _get_program()
    in_maps = make_in_maps(np.asarray(x, dtype=np.float32),
                           np.asarray(W_qkv, dtype=np.float32),
                           np.asarray(W_out, dtype=np.float32))
    res = run_bass_kernel_spmd(nc, in_maps, list(range(8)), trace=False)
    out = np.empty((B, S, D), dtype=np.float32)
    for b in range(B):
        out[b] = res.results[2 * b]["out"] + res.results[2 * b + 1]["out"]
    return out



# revision 3
# speedup vs baseline: 1.5030x; 1.5030x over previous
"""Causal self-attention (GQA + RMS-norm + RoPE) Trainium2 Bass kernel.

Sharding: 8 cores = 4 batches x 2 head-groups (tensor-parallel over heads).
Core c = 2*b + t handles batch b with Q heads [8t, 8t+8) and KV heads
[2t, 2t+2). Each core computes a partial output projection (its heads'
rows of W_out); the host sums the two partials per batch.

All matmul operands are bf16 (fp32 PSUM accumulation); elementwise /
normalization math stays fp32. Everything (qT, kT, v, ytn) is SBUF
resident -- no DRAM scratch round-trip.

Pipeline per core:
  P1: qkv = x @ W_shard (transposed-x input), RMS+RoPE on q/k in natural
      layout, PE-transpose q/k to [d, tok] into resident SBUF.
  P2: per 512-token query window, per head: scoresT = kT_tile.T @ qT_win,
      +tri-mask on diagonal tiles, exp (ACT, scale=hd^-0.5), then
      yT += v_tile.T @ expT; per-window sums for all 8 heads accumulate
      into one [8,512] PSUM bank via one-hot stationary columns; a single
      Ln/Exp pair per window gives 1/sums, broadcast via one-hot-row
      matmuls, applied on DVE.
  P3: out = sum_h yT_norm_h.T @ W_out_h rows -> partial [S, D] (bf16).

Activation-table note: all ACT functions used (Square/Ln/Exp/Copy) live
in the 'natural_log_exp_and_others' table; we reorder the table list fed
to the act-table-load pass so that table is preferred (and remap the
emitted ids back to act_info.json order), avoiding per-activation table
reloads.
"""
import sys, os
sys.path.insert(0, '/opt/trn_rl_repo')
import numpy as np
import ml_dtypes

import concourse.bacc as bacc_mod
from concourse import bass, bacc, mybir, tile

f32 = mybir.dt.float32
bf16 = mybir.dt.bfloat16
BF = ml_dtypes.bfloat16

B, S, D = 4, 2048, 2048
H, HKV, HD = 16, 4, 128
HLOC = H // 2          # 8 q heads per core
KVLOC = HKV // 2       # 2 kv heads per core
SCALE = float(HD) ** -0.5
RMS_EPS = float(np.finfo(np.float32).eps)
ROPE_BASE = 10000.0

NTC = S // 128         # 16 token tiles
NDT = D // 128         # 16 contraction tiles
NWIN = S // 512        # 4 query windows


# ---- activation-table preference patch: prefer the table holding
# square+ln+exp+copy so the kernel needs a single ACT_TABLE_LOAD. ----
_PREF_TABLE = 'natural_log_exp_and_others'
_orig_insert_atl = bacc_mod._bass_rust.insert_act_table_loads


def _patched_insert_atl(bobj, tables):
    names = [t[0] for t in tables]
    if _PREF_TABLE in names:
        pi = names.index(_PREF_TABLE)
        order = [pi] + [i for i in range(len(tables)) if i != pi]
        _orig_insert_atl(bobj, [tables[i] for i in order])
        for blk in bobj.main_func.blocks:
            for inst in blk.instructions:
                if type(inst).__name__ == 'InstLoadActFuncSet':
                    inst.act_func_set_id = order[inst.act_func_set_id]
    else:
        _orig_insert_atl(bobj, tables)


bacc_mod._bass_rust.insert_act_table_loads = _patched_insert_atl


def _rope_tables():
    inv_freq = (1.0 / (ROPE_BASE ** (np.arange(0, HD, 2, dtype=np.float32) / HD))).astype(np.float32)
    freqs = np.arange(S, dtype=np.float32)[:, None] * inv_freq[None, :]
    cos = np.cos(freqs).astype(np.float32)
    sin = np.sin(freqs).astype(np.float32)
    cos2 = np.concatenate([cos, cos], axis=1)        # [S, 128]
    sin2 = np.concatenate([sin, -sin], axis=1)       # [S, 128]
    return cos2, sin2


def _tri_masks():
    # mask[vi][p, f] = -1e30 where kv > q for scoresT diag tiles:
    # kv = 128*j + p, q = 512*w + f, vi = j - 4*w -> masked iff p + 128*vi > f
    m = np.zeros((4, 128, 512), dtype=np.float32)
    p = np.arange(128)[:, None]
    f = np.arange(512)[None, :]
    for vi in range(4):
        m[vi][(p + 128 * vi) > f] = -1e30
    return m


def _emit_rms_rope(nc, scr, psum_ap, nheads, cos1, sin1, nat_tile, eps_ap):
    """psum_ap: [128, nheads*128] qkv psum slice; writes RMS+RoPE result into
    nat_tile (SBUF, bf16). cos1/sin1: [128, 1, 128] APs (cos dup, [sin,-sin]).

    rot(q) = q*cos2 + swap_halves(q)*sin2;  out = rot(q) * rsqrt(mean(q^2)+eps)
    rsqrt computed as exp(-0.5*ln(ss/128+eps)) on ACT.
    """
    w = nheads * 128
    sq = scr.tile([128, w], bf16, tag="sq")
    nc.scalar.activation(sq[:], psum_ap, mybir.ActivationFunctionType.Square)
    ss = scr.tile([128, nheads, 1], f32, tag="ss")
    nc.vector.tensor_reduce(
        ss[:], sq[:].rearrange("p (h f) -> p h f", h=nheads),
        axis=mybir.AxisListType.X, op=mybir.AluOpType.add)
    lg = scr.tile([128, nheads, 1], f32, tag="lg")
    nc.scalar.activation(lg[:], ss[:], mybir.ActivationFunctionType.Ln,
                         scale=1.0 / HD, bias=eps_ap)
    rinv = scr.tile([128, nheads, 1, 1], f32, tag="rinv")
    nc.scalar.activation(rinv[:], lg[:], mybir.ActivationFunctionType.Exp,
                         scale=-0.5)

    shp = [128, nheads, 2, 64]
    p4 = psum_ap.rearrange("p (h x f) -> p h x f", h=nheads, x=2)
    p4s = p4[:, :, ::-1, :]
    cb = cos1.rearrange("p t (x f) -> p t x f", x=2).to_broadcast(shp)
    sb_ = sin1.rearrange("p t (x f) -> p t x f", x=2).to_broadcast(shp)
    rb = rinv[:].to_broadcast(shp)
    t1 = scr.tile(shp, bf16, tag="t1")
    t2 = scr.tile(shp, bf16, tag="t2")
    nc.vector.tensor_mul(t1[:], p4, cb)
    nc.vector.tensor_mul(t2[:], p4s, sb_)
    nc.vector.tensor_add(t1[:], t1[:], t2[:])
    nc.vector.tensor_mul(nat_tile[:].rearrange("p (h x f) -> p h x f", h=nheads, x=2),
                         t1[:], rb)


def build_program():
    cos_np, sin_np = _rope_tables()
    masks_np = _tri_masks()
    # one-hot column matrices for batched sums: oh8[p, h, c] = (c == h)
    oh_col = np.zeros((128, HLOC, HLOC), dtype=np.float32)
    for h in range(HLOC):
        oh_col[:, h, h] = 1.0
    # one-hot row matrices for broadcast: ohr[p, h, c] = (p == h)
    oh_row = np.zeros((HLOC, HLOC, 128), dtype=np.float32)
    for h in range(HLOC):
        oh_row[h, h, :] = 1.0

    nc = bacc.Bacc(trn_type="TRN2")

    xt_d = nc.dram_tensor("xt", [D, S], bf16, kind="ExternalInput")
    wq_d = nc.dram_tensor("wq", [D, HLOC * HD], bf16, kind="ExternalInput")
    wkv_d = nc.dram_tensor("wkv", [D, 2 * KVLOC * HD], bf16, kind="ExternalInput")
    wo_d = nc.dram_tensor("wo", [HLOC * HD, D], bf16, kind="ExternalInput")
    out_d = nc.dram_tensor("out", [S, D], bf16, kind="ExternalOutput")

    cos_d = nc.inline_tensor(cos_np.astype(BF), "cos_t")
    sin_d = nc.inline_tensor(sin_np.astype(BF), "sin_t")
    ident_d = nc.inline_tensor(np.eye(128, dtype=np.float32).astype(BF), "ident")
    masks_d = nc.inline_tensor(masks_np.astype(BF), "tri_masks")
    ohc_d = nc.inline_tensor(oh_col.astype(BF), "oh_col")
    ohr_d = nc.inline_tensor(oh_row.astype(BF), "oh_row")

    with tile.TileContext(nc) as tc:
        with tc.tile_pool(name="cst", bufs=1) as cst:
            cos_sb = cst.tile([128, NTC, 128], bf16, tag="cos")
            sin_sb = cst.tile([128, NTC, 128], bf16, tag="sin")
            ident = cst.tile([128, 128], bf16, tag="ident")
            masks = cst.tile([128, 4, 512], bf16, tag="masks")
            ohc = cst.tile([128, HLOC, HLOC], bf16, tag="ohc")
            ohr = cst.tile([HLOC, HLOC, 128], bf16, tag="ohr")
            eps_sb = cst.tile([128, 1], f32, tag="eps")
            nc.sync.dma_start(out=cos_sb[:], in_=cos_d[:].rearrange("(t p) f -> p t f", p=128))
            nc.sync.dma_start(out=sin_sb[:], in_=sin_d[:].rearrange("(t p) f -> p t f", p=128))
            nc.sync.dma_start(out=ident[:], in_=ident_d[:])
            nc.sync.dma_start(out=masks[:], in_=masks_d[:].rearrange("v p f -> p v f"))
            nc.sync.dma_start(out=ohc[:], in_=ohc_d[:])
            nc.sync.dma_start(out=ohr[:], in_=ohr_d[:])
            nc.gpsimd.memset(eps_sb[:], RMS_EPS)

            # resident tensors
            qt_sb = cst.tile([128, HLOC, S], bf16, tag="qt")
            kt_sb = cst.tile([128, KVLOC, S], bf16, tag="kt")
            v_sb = cst.tile([128, NTC, KVLOC * HD], bf16, tag="v")
            ytn = cst.tile([128, HLOC, S], bf16, tag="ytn")
            wq_sb = cst.tile([128, NDT, HLOC * HD], bf16, tag="wq")
            wkv_sb = cst.tile([128, NDT, 512], bf16, tag="wkv")
            wo_sb = cst.tile([128, HLOC, D], bf16, tag="wo")

            # weight preloads (sync queue; single large DMAs)
            nc.sync.dma_start(out=wkv_sb[:], in_=wkv_d[:].rearrange("(t p) c -> p t c", p=128))
            nc.sync.dma_start(out=wq_sb[:], in_=wq_d[:].rearrange("(t p) c -> p t c", p=128))
            nc.sync.dma_start(out=wo_sb[:], in_=wo_d[:].rearrange("(h p) c -> p h c", p=128))

            # ---------------- Phase 1: QKV projection ----------------
            with tc.tile_pool(name="xs", bufs=2) as xs, \
                 tc.tile_pool(name="nat", bufs=2) as nat, \
                 tc.tile_pool(name="p1a", bufs=6, space="PSUM") as p1a, \
                 tc.tile_pool(name="p1t", bufs=2, space="PSUM") as p1t:

                for tcid in range(NTC):
                    xt_sb = xs.tile([128, NDT, 128], bf16, tag="xt")
                    nc.sync.dma_start(
                        out=xt_sb[:],
                        in_=xt_d[:, tcid * 128:(tcid + 1) * 128]
                            .rearrange("(t p) s -> p t s", p=128))

                    ps_q1 = p1a.tile([128, 512], f32, tag="acc")
                    ps_q2 = p1a.tile([128, 512], f32, tag="acc")
                    ps_kv = p1a.tile([128, 512], f32, tag="acc")
                    for dt in range(NDT):
                        st, sp = dt == 0, dt == NDT - 1
                        nc.tensor.matmul(ps_kv[:], xt_sb[:, dt, :], wkv_sb[:, dt, :], start=st, stop=sp)
                    for dt in range(NDT):
                        st, sp = dt == 0, dt == NDT - 1
                        lhs = xt_sb[:, dt, :]
                        nc.tensor.matmul(ps_q1[:], lhs, wq_sb[:, dt, 0:512], start=st, stop=sp)
                        nc.tensor.matmul(ps_q2[:], lhs, wq_sb[:, dt, 512:1024], start=st, stop=sp)

                    cos1 = cos_sb[:, tcid:tcid + 1, :]
                    sin1 = sin_sb[:, tcid:tcid + 1, :]

                    # q heads 0-3 / 4-7: RMS+RoPE, then PE-transpose to qT
                    for gi, ps in ((0, ps_q1), (1, ps_q2)):
                        qn = nat.tile([128, 512], bf16, tag="qn")
                        _emit_rms_rope(nc, nat, ps[:], 4, cos1, sin1, qn, eps_sb[:])
                        for hh in range(4):
                            h = gi * 4 + hh
                            tp = p1t.tile([128, 128], bf16, tag="tp")
                            nc.tensor.transpose(tp[:], qn[:, hh * 128:(hh + 1) * 128], ident[:])
                            if hh % 2 == 0:
                                nc.vector.tensor_copy(qt_sb[:, h, tcid * 128:(tcid + 1) * 128], tp[:])
                            else:
                                nc.scalar.activation(qt_sb[:, h, tcid * 128:(tcid + 1) * 128], tp[:],
                                                     mybir.ActivationFunctionType.Copy)

                    # k heads (cols 0:256 of kv psum)
                    kn = nat.tile([128, 256], bf16, tag="kn")
                    _emit_rms_rope(nc, nat, ps_kv[:, 0:256], 2, cos1, sin1, kn, eps_sb[:])
                    for kh in range(KVLOC):
                        tp = p1t.tile([128, 128], bf16, tag="tp")
                        nc.tensor.transpose(tp[:], kn[:, kh * 128:(kh + 1) * 128], ident[:])
                        if kh % 2 == 0:
                            nc.vector.tensor_copy(kt_sb[:, kh, tcid * 128:(tcid + 1) * 128], tp[:])
                        else:
                            nc.scalar.activation(kt_sb[:, kh, tcid * 128:(tcid + 1) * 128], tp[:],
                                                 mybir.ActivationFunctionType.Copy)

                    # v: plain copy out (natural layout)
                    nc.vector.tensor_copy(v_sb[:, tcid, :], ps_kv[:, 256:512])

            # ---- Phase 2: attention ----
            with tc.tile_pool(name="ex", bufs=4) as ex, \
                 tc.tile_pool(name="sm", bufs=2) as sm, \
                 tc.tile_pool(name="p2s", bufs=3, space="PSUM") as p2s, \
                 tc.tile_pool(name="p2y", bufs=2, space="PSUM") as p2y, \
                 tc.tile_pool(name="p2n", bufs=1, space="PSUM") as p2n, \
                 tc.tile_pool(name="p2b", bufs=2, space="PSUM") as p2b:

                for w in range(NWIN):
                    njt = 4 * w + 4
                    ps_sums = p2n.tile([HLOC, 512], f32, tag="sums")
                    for hq in range(HLOC):
                        kvh = hq // 4
                        ps_y = p2y.tile([128, 512], f32, tag="y")
                        rhs_q = qt_sb[:, hq, w * 512:(w + 1) * 512]
                        for j in range(njt):
                            ps_sc = p2s.tile([128, 512], f32, tag="sc")
                            nc.tensor.matmul(
                                ps_sc[:],
                                kt_sb[:, kvh, j * 128:(j + 1) * 128],
                                rhs_q)
                            if j >= 4 * w:
                                nc.vector.tensor_add(ps_sc[:], ps_sc[:], masks[:, j - 4 * w, :])
                            et = ex.tile([128, 512], bf16, tag="et")
                            nc.scalar.activation(et[:], ps_sc[:],
                                                 mybir.ActivationFunctionType.Exp,
                                                 scale=SCALE)
                            st, sp = j == 0, j == njt - 1
                            nc.tensor.matmul(
                                ps_y[:],
                                v_sb[:, j, kvh * 128:(kvh + 1) * 128],
                                et[:], start=st, stop=sp,
                                skip_group_check=True)
                            nc.tensor.matmul(
                                ps_sums[:], ohc[:, hq, :], et[:],
                                start=(hq == 0 and j == 0),
                                stop=(hq == HLOC - 1 and j == njt - 1),
                                skip_group_check=True)
                        # stash raw yT (normalized later this window)
                        nc.vector.tensor_copy(ytn[:, hq, w * 512:(w + 1) * 512], ps_y[:])

                    # batched 1/sums for all 8 heads of this window
                    lgs = sm.tile([HLOC, 512], f32, tag="lgs")
                    nc.scalar.activation(lgs[:], ps_sums[:],
                                         mybir.ActivationFunctionType.Ln)
                    rec = sm.tile([HLOC, 512], bf16, tag="rec")
                    nc.scalar.activation(rec[:], lgs[:],
                                         mybir.ActivationFunctionType.Exp,
                                         scale=-1.0)
                    for hq in range(HLOC):
                        bcp = p2b.tile([128, 512], f32, tag="bc")
                        nc.tensor.matmul(bcp[:], ohr[:, hq, :], rec[:])
                        nc.vector.tensor_mul(
                            ytn[:, hq, w * 512:(w + 1) * 512],
                            ytn[:, hq, w * 512:(w + 1) * 512], bcp[:])

            # ---- Phase 3: output projection ----
            with tc.tile_pool(name="ob", bufs=2) as ob, \
                 tc.tile_pool(name="p3", bufs=4, space="PSUM") as p3:
                for og in range(4):
                    for tcid in range(NTC):
                        ps_o = p3.tile([128, 512], f32, tag="o")
                        for h in range(HLOC):
                            nc.tensor.matmul(
                                ps_o[:],
                                ytn[:, h, tcid * 128:(tcid + 1) * 128],
                                wo_sb[:, h, og * 512:(og + 1) * 512],
                                start=(h == 0), stop=(h == HLOC - 1))
                        ot = ob.tile([128, 512], bf16, tag="ot")
                        nc.scalar.activation(ot[:], ps_o[:],
                                             mybir.ActivationFunctionType.Copy)
                        nc.gpsimd.dma_start(
                            out=out_d[tcid * 128:(tcid + 1) * 128, og * 512:(og + 1) * 512],
                            in_=ot[:])

    nc.compile()
    return nc


_PROGRAM = None


def _get_program():
    global _PROGRAM
    if _PROGRAM is None:
        _PROGRAM = build_program()
    return _PROGRAM


def make_in_maps(x, W_qkv, W_out):
    in_maps = []
    for c in range(8):
        b, t = c // 2, c % 2
        xt = np.ascontiguousarray(x[b].T).astype(BF)
        wq = np.ascontiguousarray(W_qkv[:, t * 1024:(t + 1) * 1024]).astype(BF)
        wk = W_qkv[:, D + t * 256: D + (t + 1) * 256]
        wv = W_qkv[:, D + 512 + t * 256: D + 512 + (t + 1) * 256]
        wkv = np.ascontiguousarray(np.concatenate([wk, wv], axis=1)).astype(BF)
        wo = np.ascontiguousarray(W_out[t * 1024:(t + 1) * 1024, :]).astype(BF)
        in_maps.append({"xt": xt, "wq": wq, "wkv": wkv, "wo": wo})
    return in_maps


def kernel(x, W_qkv, W_out):
    from concourse.bass_utils import run_bass_kernel_spmd
    nc = _get_program()
    in_maps = make_in_maps(np.asarray(x, dtype=np.float32),
                           np.asarray(W_qkv, dtype=np.float32),
                           np.asarray(W_out, dtype=np.float32))
    res = run_bass_kernel_spmd(nc, in_maps, list(range(8)), trace=False)
    out = np.empty((B, S, D), dtype=np.float32)
    for b in range(B):
        out[b] = (res.results[2 * b]["out"].astype(np.float32)
                  + res.results[2 * b + 1]["out"].astype(np.float32))
    return out


# revision 6
# speedup vs baseline: 1.5652x; 1.0414x over previous
"""Causal self-attention (GQA + RMS-norm + RoPE) Trainium2 Bass kernel.

Sharding: 8 cores = 4 batches x 2 head-groups (tensor-parallel over heads).
Core c = 2*b + t handles batch b with Q heads [8t, 8t+8) and KV heads
[2t, 2t+2). Each core computes a partial output projection (its heads'
rows of W_out); the host sums the two partials per batch.

All matmul operands are bf16 (fp32 PSUM accumulation); elementwise /
normalization math stays fp32. Everything (qT, kT, v, ytn) is SBUF
resident -- no DRAM scratch round-trip.

Pipeline per core:
  P1: qkv = x @ W_shard (transposed-x input), RMS+RoPE on q/k in natural
      layout, PE-transpose q/k to [d, tok] into resident SBUF.
  P2: per 512-token query window, per head: scoresT = kT_tile.T @ qT_win,
      +tri-mask on diagonal tiles, exp (ACT, scale=hd^-0.5), then
      yT += v_tile.T @ expT; per-window sums for all 8 heads accumulate
      into one [8,512] PSUM bank via one-hot stationary columns; a single
      Ln/Exp pair per window gives 1/sums, broadcast via one-hot-row
      matmuls, applied on DVE.
  P3: out = sum_h yT_norm_h.T @ W_out_h rows -> partial [S, D] (bf16).

Activation-table note: all ACT functions used (Square/Ln/Exp/Copy) live
in the 'natural_log_exp_and_others' table; we reorder the table list fed
to the act-table-load pass so that table is preferred (and remap the
emitted ids back to act_info.json order), avoiding per-activation table
reloads.
"""
import sys, os
sys.path.insert(0, '/opt/trn_rl_repo')
import numpy as np
import ml_dtypes

import concourse.bacc as bacc_mod
from concourse import bass, bacc, mybir, tile

f32 = mybir.dt.float32
bf16 = mybir.dt.bfloat16
BF = ml_dtypes.bfloat16

B, S, D = 4, 2048, 2048
H, HKV, HD = 16, 4, 128
HLOC = H // 2          # 8 q heads per core
KVLOC = HKV // 2       # 2 kv heads per core
SCALE = float(HD) ** -0.5
RMS_EPS = float(np.finfo(np.float32).eps)
ROPE_BASE = 10000.0

NTC = S // 128         # 16 token tiles
NDT = D // 128         # 16 contraction tiles
NWIN = S // 512        # 4 query windows


# ---- activation-table preference patch: prefer the table holding
# square+ln+exp+copy so the kernel needs a single ACT_TABLE_LOAD. ----
_PREF_TABLE = 'natural_log_exp_and_others'
_orig_insert_atl = bacc_mod._bass_rust.insert_act_table_loads


def _patched_insert_atl(bobj, tables):
    names = [t[0] for t in tables]
    if _PREF_TABLE in names:
        pi = names.index(_PREF_TABLE)
        order = [pi] + [i for i in range(len(tables)) if i != pi]
        _orig_insert_atl(bobj, [tables[i] for i in order])
        for blk in bobj.main_func.blocks:
            for inst in blk.instructions:
                if type(inst).__name__ == 'InstLoadActFuncSet':
                    inst.act_func_set_id = order[inst.act_func_set_id]
    else:
        _orig_insert_atl(bobj, tables)


bacc_mod._bass_rust.insert_act_table_loads = _patched_insert_atl


def _rope_tables():
    inv_freq = (1.0 / (ROPE_BASE ** (np.arange(0, HD, 2, dtype=np.float32) / HD))).astype(np.float32)
    freqs = np.arange(S, dtype=np.float32)[:, None] * inv_freq[None, :]
    cos = np.cos(freqs).astype(np.float32)
    sin = np.sin(freqs).astype(np.float32)
    cos2 = np.concatenate([cos, cos], axis=1)        # [S, 128]
    sin2 = np.concatenate([sin, -sin], axis=1)       # [S, 128]
    return cos2, sin2


def _tri_masks():
    # triangular mask for the 128-col diagonal block of a diag score tile:
    # local col f, row p: masked iff p > f
    m = np.zeros((128, 128), dtype=np.float32)
    p = np.arange(128)[:, None]
    f = np.arange(128)[None, :]
    m[p > f] = -1e30
    return m


def _emit_rms_rope(nc, scr, psum_ap, nheads, cos1, sin1, nat_tile, eps_ap):
    """psum_ap: [128, nheads*128] qkv psum slice; writes RMS+RoPE result into
    nat_tile (SBUF, bf16). cos1/sin1: [128, 1, 128] APs (cos dup, [sin,-sin]).

    rot(q) = q*cos2 + swap_halves(q)*sin2;  out = rot(q) * rsqrt(mean(q^2)+eps)
    rsqrt computed as exp(-0.5*ln(ss/128+eps)) on ACT.
    """
    w = nheads * 128
    sq = scr.tile([128, w], bf16, tag="sq")
    nc.scalar.activation(sq[:], psum_ap, mybir.ActivationFunctionType.Square)
    ss = scr.tile([128, nheads, 1], f32, tag="ss")
    nc.vector.tensor_reduce(
        ss[:], sq[:].rearrange("p (h f) -> p h f", h=nheads),
        axis=mybir.AxisListType.X, op=mybir.AluOpType.add)
    lg = scr.tile([128, nheads, 1], f32, tag="lg")
    nc.scalar.activation(lg[:], ss[:], mybir.ActivationFunctionType.Ln,
                         scale=1.0 / HD, bias=eps_ap)
    rinv = scr.tile([128, nheads, 1, 1], f32, tag="rinv")
    nc.scalar.activation(rinv[:], lg[:], mybir.ActivationFunctionType.Exp,
                         scale=-0.5)

    shp = [128, nheads, 2, 64]
    p4 = psum_ap.rearrange("p (h x f) -> p h x f", h=nheads, x=2)
    p4s = p4[:, :, ::-1, :]
    cb = cos1.rearrange("p t (x f) -> p t x f", x=2).to_broadcast(shp)
    sb_ = sin1.rearrange("p t (x f) -> p t x f", x=2).to_broadcast(shp)
    rb = rinv[:].to_broadcast(shp)
    t1 = scr.tile(shp, bf16, tag="t1")
    t2 = scr.tile(shp, bf16, tag="t2")
    nc.vector.tensor_mul(t1[:], p4, cb)
    nc.vector.tensor_mul(t2[:], p4s, sb_)
    nc.vector.tensor_add(t1[:], t1[:], t2[:])
    nc.vector.tensor_mul(nat_tile[:].rearrange("p (h x f) -> p h x f", h=nheads, x=2),
                         t1[:], rb)


def build_program():
    cos_np, sin_np = _rope_tables()
    masks_np = _tri_masks()
    # one-hot column matrices for batched sums: oh8[p, h, c] = (c == h)
    oh_col = np.zeros((128, HLOC, HLOC), dtype=np.float32)
    for h in range(HLOC):
        oh_col[:, h, h] = 1.0
    # one-hot row matrices for broadcast: ohr[p, h, c] = (p == h)
    oh_row = np.zeros((HLOC, HLOC, 128), dtype=np.float32)
    for h in range(HLOC):
        oh_row[h, h, :] = 1.0

    nc = bacc.Bacc(trn_type="TRN2")

    xt_d = nc.dram_tensor("xt", [D, S], bf16, kind="ExternalInput")
    wq_d = nc.dram_tensor("wq", [D, HLOC * HD], bf16, kind="ExternalInput")
    wkv_d = nc.dram_tensor("wkv", [D, 2 * KVLOC * HD], bf16, kind="ExternalInput")
    wo_d = nc.dram_tensor("wo", [HLOC * HD, D], bf16, kind="ExternalInput")
    out_d = nc.dram_tensor("out", [S, D], bf16, kind="ExternalOutput")

    cos_d = nc.inline_tensor(cos_np.astype(BF), "cos_t")
    sin_d = nc.inline_tensor(sin_np.astype(BF), "sin_t")
    ident_d = nc.inline_tensor(np.eye(128, dtype=np.float32).astype(BF), "ident")
    masks_d = nc.inline_tensor(masks_np.astype(BF), "tri_masks")
    ohc_d = nc.inline_tensor(oh_col.astype(BF), "oh_col")
    ohr_d = nc.inline_tensor(oh_row.astype(BF), "oh_row")

    with tile.TileContext(nc) as tc:
        with tc.tile_pool(name="cst", bufs=1) as cst:
            cos_sb = cst.tile([128, NTC, 128], bf16, tag="cos")
            sin_sb = cst.tile([128, NTC, 128], bf16, tag="sin")
            ident = cst.tile([128, 128], bf16, tag="ident")
            masks = cst.tile([128, 128], bf16, tag="masks")
            ohc = cst.tile([128, HLOC, HLOC], bf16, tag="ohc")
            ohr = cst.tile([HLOC, HLOC, 128], bf16, tag="ohr")
            eps_sb = cst.tile([128, 1], f32, tag="eps")
            nc.sync.dma_start(out=cos_sb[:], in_=cos_d[:].rearrange("(t p) f -> p t f", p=128))
            nc.sync.dma_start(out=sin_sb[:], in_=sin_d[:].rearrange("(t p) f -> p t f", p=128))
            nc.gpsimd.dma_start(out=ident[:], in_=ident_d[:])
            nc.gpsimd.dma_start(out=masks[:], in_=masks_d[:])
            nc.gpsimd.dma_start(out=ohc[:], in_=ohc_d[:])
            nc.gpsimd.dma_start(out=ohr[:], in_=ohr_d[:])
            nc.gpsimd.memset(eps_sb[:], RMS_EPS)

            # resident tensors
            qt_sb = cst.tile([128, HLOC, S], bf16, tag="qt")
            kt_sb = cst.tile([128, KVLOC, S], bf16, tag="kt")
            v_sb = cst.tile([128, NTC, KVLOC * HD], bf16, tag="v")
            ytn = cst.tile([128, HLOC, S], bf16, tag="ytn")
            wq_sb = cst.tile([128, NDT, HLOC * HD], bf16, tag="wq")
            wkv_sb = cst.tile([128, NDT, 512], bf16, tag="wkv")
            wo_sb = cst.tile([128, HLOC, D], bf16, tag="wo")

            # weight preloads: per-slice, spread across idle queues so the
            # first matmuls can start as soon as their slices land
            wkv_r = wkv_d[:].rearrange("(t p) c -> p t c", p=128)
            wq_r = wq_d[:].rearrange("(t p) c -> p t c", p=128)
            wo_r = wo_d[:].rearrange("(h p) c -> p h c", p=128)
            for dt in range(NDT):
                nc.sync.dma_start(out=wkv_sb[:, dt, :], in_=wkv_r[:, dt, :])
                nc.scalar.dma_start(out=wq_sb[:, dt, :], in_=wq_r[:, dt, :])
            for h in range(HLOC):
                nc.gpsimd.dma_start(out=wo_sb[:, h, :], in_=wo_r[:, h, :])

            # ---------------- Phase 1: QKV projection ----------------
            with tc.tile_pool(name="xs", bufs=2) as xs, \
                 tc.tile_pool(name="nat", bufs=2) as nat, \
                 tc.tile_pool(name="p1a", bufs=6, space="PSUM") as p1a, \
                 tc.tile_pool(name="p1t", bufs=2, space="PSUM") as p1t:

                for tcid in range(NTC):
                    xt_sb = xs.tile([128, NDT, 128], bf16, tag="xt")
                    nc.sync.dma_start(
                        out=xt_sb[:],
                        in_=xt_d[:, tcid * 128:(tcid + 1) * 128]
                            .rearrange("(t p) s -> p t s", p=128))

                    ps_q1 = p1a.tile([128, 512], f32, tag="acc")
                    ps_q2 = p1a.tile([128, 512], f32, tag="acc")
                    ps_kv = p1a.tile([128, 512], f32, tag="acc")
                    for dt in range(NDT):
                        st, sp = dt == 0, dt == NDT - 1
                        nc.tensor.matmul(ps_kv[:], xt_sb[:, dt, :], wkv_sb[:, dt, :], start=st, stop=sp)
                    for dt in range(NDT):
                        st, sp = dt == 0, dt == NDT - 1
                        lhs = xt_sb[:, dt, :]
                        nc.tensor.matmul(ps_q1[:], lhs, wq_sb[:, dt, 0:512], start=st, stop=sp)
                        nc.tensor.matmul(ps_q2[:], lhs, wq_sb[:, dt, 512:1024], start=st, stop=sp)

                    cos1 = cos_sb[:, tcid:tcid + 1, :]
                    sin1 = sin_sb[:, tcid:tcid + 1, :]

                    # q heads 0-3 / 4-7: RMS+RoPE, then PE-transpose to qT
                    for gi, ps in ((0, ps_q1), (1, ps_q2)):
                        qn = nat.tile([128, 512], bf16, tag="qn")
                        _emit_rms_rope(nc, nat, ps[:], 4, cos1, sin1, qn, eps_sb[:])
                        for hh in range(4):
                            h = gi * 4 + hh
                            tp = p1t.tile([128, 128], bf16, tag="tp")
                            nc.tensor.transpose(tp[:], qn[:, hh * 128:(hh + 1) * 128], ident[:])
                            if hh % 2 == 0:
                                nc.vector.tensor_copy(qt_sb[:, h, tcid * 128:(tcid + 1) * 128], tp[:])
                            else:
                                nc.scalar.activation(qt_sb[:, h, tcid * 128:(tcid + 1) * 128], tp[:],
                                                     mybir.ActivationFunctionType.Copy)

                    # k heads (cols 0:256 of kv psum)
                    kn = nat.tile([128, 256], bf16, tag="kn")
                    _emit_rms_rope(nc, nat, ps_kv[:, 0:256], 2, cos1, sin1, kn, eps_sb[:])
                    for kh in range(KVLOC):
                        tp = p1t.tile([128, 128], bf16, tag="tp")
                        nc.tensor.transpose(tp[:], kn[:, kh * 128:(kh + 1) * 128], ident[:])
                        if kh % 2 == 0:
                            nc.vector.tensor_copy(kt_sb[:, kh, tcid * 128:(tcid + 1) * 128], tp[:])
                        else:
                            nc.scalar.activation(kt_sb[:, kh, tcid * 128:(tcid + 1) * 128], tp[:],
                                                 mybir.ActivationFunctionType.Copy)

                    # v: plain copy out (natural layout)
                    nc.vector.tensor_copy(v_sb[:, tcid, :], ps_kv[:, 256:512])

            # ---- Phase 2: attention ----
            # Heads processed in pairs with interleaved j-chains so PE always
            # has independent work while exp/mask complete. Diagonal score
            # tiles use partial-width MMs/exp (columns below the diagonal
            # block are fully masked and simply not computed); only the
            # [128,128] diagonal block needs the additive triangular mask.
            with tc.tile_pool(name="ex", bufs=6) as ex, \
                 tc.tile_pool(name="sm", bufs=2) as sm, \
                 tc.tile_pool(name="p2s", bufs=4, space="PSUM") as p2s, \
                 tc.tile_pool(name="p2y", bufs=2, space="PSUM") as p2y, \
                 tc.tile_pool(name="p2n", bufs=1, space="PSUM") as p2n:

                for w in range(NWIN):
                    njt = 4 * w + 4
                    ps_sums = p2n.tile([HLOC, 512], f32, tag="sums")
                    for hp in range(HLOC // 2):
                        h0, h1 = 2 * hp, 2 * hp + 1
                        kvh = h0 // 4
                        ps_y0 = p2y.tile([128, 512], f32, tag="y")
                        ps_y1 = p2y.tile([128, 512], f32, tag="y")
                        for j in range(njt):
                            vi = j - 4 * w
                            c0 = 128 * vi if vi >= 0 else 0
                            kt_j = kt_sb[:, kvh, j * 128:(j + 1) * 128]
                            v_j = v_sb[:, j, kvh * 128:(kvh + 1) * 128]
                            st, sp = j == 0, j == njt - 1
                            for hq, ps_y in ((h0, ps_y0), (h1, ps_y1)):
                                ps_sc = p2s.tile([128, 512], f32, tag="sc")
                                nc.tensor.matmul(
                                    ps_sc[:, c0:512], kt_j,
                                    qt_sb[:, hq, w * 512 + c0:(w + 1) * 512])
                                if vi >= 0:
                                    nc.vector.tensor_add(ps_sc[:, c0:c0 + 128],
                                                         ps_sc[:, c0:c0 + 128],
                                                         masks[:])
                                et = ex.tile([128, 512], bf16, tag="et")
                                nc.scalar.activation(et[:, c0:512], ps_sc[:, c0:512],
                                                     mybir.ActivationFunctionType.Exp,
                                                     scale=SCALE)
                                nc.tensor.matmul(
                                    ps_y[:, c0:512], v_j,
                                    et[:, c0:512], start=st, stop=sp,
                                    skip_group_check=True)
                                nc.tensor.matmul(
                                    ps_sums[:, c0:512], ohc[:, hq, :], et[:, c0:512],
                                    start=(hq == 0 and j == 0),
                                    stop=(hq == HLOC - 1 and j == njt - 1),
                                    skip_group_check=True)
                        # stash raw yT (normalized later this window)
                        nc.vector.tensor_copy(ytn[:, h0, w * 512:(w + 1) * 512], ps_y0[:])
                        nc.vector.tensor_copy(ytn[:, h1, w * 512:(w + 1) * 512], ps_y1[:])

                    # batched 1/sums for all 8 heads of this window
                    lgs = sm.tile([HLOC, 512], f32, tag="lgs")
                    nc.scalar.activation(lgs[:], ps_sums[:],
                                         mybir.ActivationFunctionType.Ln)
                    rec = sm.tile([HLOC, 512], bf16, tag="rec")
                    nc.scalar.activation(rec[:], lgs[:],
                                         mybir.ActivationFunctionType.Exp,
                                         scale=-1.0)
                    for hq in range(HLOC):
                        bcp = p2s.tile([128, 512], f32, tag="sc")
                        nc.tensor.matmul(bcp[:], ohr[:, hq, :], rec[:])
                        nc.vector.tensor_mul(
                            ytn[:, hq, w * 512:(w + 1) * 512],
                            ytn[:, hq, w * 512:(w + 1) * 512], bcp[:])

            # ---- Phase 3: output projection ----
            with tc.tile_pool(name="ob", bufs=2) as ob, \
                 tc.tile_pool(name="p3", bufs=4, space="PSUM") as p3:
                for og in range(4):
                    for tcid in range(NTC):
                        ps_o = p3.tile([128, 512], f32, tag="o")
                        for h in range(HLOC):
                            nc.tensor.matmul(
                                ps_o[:],
                                ytn[:, h, tcid * 128:(tcid + 1) * 128],
                                wo_sb[:, h, og * 512:(og + 1) * 512],
                                start=(h == 0), stop=(h == HLOC - 1))
                        ot = ob.tile([128, 512], bf16, tag="ot")
                        nc.scalar.activation(ot[:], ps_o[:],
                                             mybir.ActivationFunctionType.Copy)
                        nc.gpsimd.dma_start(
                            out=out_d[tcid * 128:(tcid + 1) * 128, og * 512:(og + 1) * 512],
                            in_=ot[:])

    nc.compile()
    return nc


_PROGRAM = None


def _get_program():
    global _PROGRAM
    if _PROGRAM is None:
        _PROGRAM = build_program()
    return _PROGRAM


def make_in_maps(x, W_qkv, W_out):
    in_maps = []
    for c in range(8):
        b, t = c // 2, c % 2
        xt = np.ascontiguousarray(x[b].T).astype(BF)
        wq = np.ascontiguousarray(W_qkv[:, t * 1024:(t + 1) * 1024]).astype(BF)
        wk = W_qkv[:, D + t * 256: D + (t + 1) * 256]
        wv = W_qkv[:, D + 512 + t * 256: D + 512 + (t + 1) * 256]
        wkv = np.ascontiguousarray(np.concatenate([wk, wv], axis=1)).astype(BF)
        wo = np.ascontiguousarray(W_out[t * 1024:(t + 1) * 1024, :]).astype(BF)
        in_maps.append({"xt": xt, "wq": wq, "wkv": wkv, "wo": wo})
    return in_maps


def kernel(x, W_qkv, W_out):
    from concourse.bass_utils import run_bass_kernel_spmd
    nc = _get_program()
    in_maps = make_in_maps(np.asarray(x, dtype=np.float32),
                           np.asarray(W_qkv, dtype=np.float32),
                           np.asarray(W_out, dtype=np.float32))
    res = run_bass_kernel_spmd(nc, in_maps, list(range(8)), trace=False)
    out = np.empty((B, S, D), dtype=np.float32)
    for b in range(B):
        out[b] = (res.results[2 * b]["out"].astype(np.float32)
                  + res.results[2 * b + 1]["out"].astype(np.float32))
    return out


# revision 7
# speedup vs baseline: 1.6100x; 1.0287x over previous
"""Causal self-attention (GQA + RMS-norm + RoPE) Trainium2 Bass kernel.

Sharding: 8 cores = 4 batches x 2 head-groups (tensor-parallel over heads).
Core c = 2*b + t handles batch b with Q heads [8t, 8t+8) and KV heads
[2t, 2t+2). Each core computes a partial output projection (its heads'
rows of W_out); the host sums the two partials per batch.

All matmul operands are bf16 (fp32 PSUM accumulation); elementwise /
normalization math stays fp32. Everything (qT, kT, v, ytn) is SBUF
resident -- no DRAM scratch round-trip.

Pipeline per core:
  P1: qkv = x @ W_shard (transposed-x input), RMS+RoPE on q/k in natural
      layout, PE-transpose q/k to [d, tok] into resident SBUF.
  P2: per 512-token query window, per head: scoresT = kT_tile.T @ qT_win,
      +tri-mask on diagonal tiles, exp (ACT, scale=hd^-0.5), then
      yT += v_tile.T @ expT; per-window sums for all 8 heads accumulate
      into one [8,512] PSUM bank via one-hot stationary columns; a single
      Ln/Exp pair per window gives 1/sums, broadcast via one-hot-row
      matmuls, applied on DVE.
  P3: out = sum_h yT_norm_h.T @ W_out_h rows -> partial [S, D] (bf16).

Activation-table note: all ACT functions used (Square/Ln/Exp/Copy) live
in the 'natural_log_exp_and_others' table; we reorder the table list fed
to the act-table-load pass so that table is preferred (and remap the
emitted ids back to act_info.json order), avoiding per-activation table
reloads.
"""
import sys, os
sys.path.insert(0, '/opt/trn_rl_repo')
import numpy as np
import ml_dtypes

import concourse.bacc as bacc_mod
from concourse import bass, bacc, mybir, tile

f32 = mybir.dt.float32
bf16 = mybir.dt.bfloat16
BF = ml_dtypes.bfloat16

B, S, D = 4, 2048, 2048
H, HKV, HD = 16, 4, 128
HLOC = H // 2          # 8 q heads per core
KVLOC = HKV // 2       # 2 kv heads per core
SCALE = float(HD) ** -0.5
RMS_EPS = float(np.finfo(np.float32).eps)
ROPE_BASE = 10000.0

NTC = S // 128         # 16 token tiles
NDT = D // 128         # 16 contraction tiles
NWIN = S // 512        # 4 query windows


# ---- activation-table preference patch: prefer the table holding
# square+ln+exp+copy so the kernel needs a single ACT_TABLE_LOAD. ----
_PREF_TABLE = 'natural_log_exp_and_others'
_orig_insert_atl = bacc_mod._bass_rust.insert_act_table_loads


def _patched_insert_atl(bobj, tables):
    names = [t[0] for t in tables]
    if _PREF_TABLE in names:
        pi = names.index(_PREF_TABLE)
        order = [pi] + [i for i in range(len(tables)) if i != pi]
        _orig_insert_atl(bobj, [tables[i] for i in order])
        for blk in bobj.main_func.blocks:
            for inst in blk.instructions:
                if type(inst).__name__ == 'InstLoadActFuncSet':
                    inst.act_func_set_id = order[inst.act_func_set_id]
    else:
        _orig_insert_atl(bobj, tables)


bacc_mod._bass_rust.insert_act_table_loads = _patched_insert_atl


def _rope_tables():
    inv_freq = (1.0 / (ROPE_BASE ** (np.arange(0, HD, 2, dtype=np.float32) / HD))).astype(np.float32)
    freqs = np.arange(S, dtype=np.float32)[:, None] * inv_freq[None, :]
    cos = np.cos(freqs).astype(np.float32)
    sin = np.sin(freqs).astype(np.float32)
    cos2 = np.concatenate([cos, cos], axis=1)        # [S, 128]
    sin2 = np.concatenate([sin, -sin], axis=1)       # [S, 128]
    return cos2, sin2


def _tri_masks():
    # triangular mask for the 128-col diagonal block of a diag score tile:
    # local col f, row p: masked iff p > f
    m = np.zeros((128, 128), dtype=np.float32)
    p = np.arange(128)[:, None]
    f = np.arange(128)[None, :]
    m[p > f] = -1e30
    return m


def _emit_rms_rope(nc, scr, psum_ap, nheads, cos1, sin1, nat_tile, eps_ap):
    """psum_ap: [128, nheads*128] qkv psum slice; writes RMS+RoPE result into
    nat_tile (SBUF, bf16). cos1/sin1: [128, 1, 128] APs (cos dup, [sin,-sin]).

    rot(q) = q*cos2 + swap_halves(q)*sin2;  out = rot(q) * rsqrt(mean(q^2)+eps)
    rsqrt computed as exp(-0.5*ln(ss/128+eps)) on ACT.
    """
    w = nheads * 128
    sq = scr.tile([128, w], bf16, tag="sq")
    nc.scalar.activation(sq[:], psum_ap, mybir.ActivationFunctionType.Square)
    ss = scr.tile([128, nheads, 1], f32, tag="ss")
    nc.vector.tensor_reduce(
        ss[:], sq[:].rearrange("p (h f) -> p h f", h=nheads),
        axis=mybir.AxisListType.X, op=mybir.AluOpType.add)
    lg = scr.tile([128, nheads, 1], f32, tag="lg")
    nc.scalar.activation(lg[:], ss[:], mybir.ActivationFunctionType.Ln,
                         scale=1.0 / HD, bias=eps_ap)
    rinv = scr.tile([128, nheads, 1, 1], f32, tag="rinv")
    nc.scalar.activation(rinv[:], lg[:], mybir.ActivationFunctionType.Exp,
                         scale=-0.5)

    shp = [128, nheads, 2, 64]
    p4 = psum_ap.rearrange("p (h x f) -> p h x f", h=nheads, x=2)
    p4s = p4[:, :, ::-1, :]
    cb = cos1.rearrange("p t (x f) -> p t x f", x=2).to_broadcast(shp)
    sb_ = sin1.rearrange("p t (x f) -> p t x f", x=2).to_broadcast(shp)
    rb = rinv[:].to_broadcast(shp)
    t1 = scr.tile(shp, bf16, tag="t1")
    t2 = scr.tile(shp, bf16, tag="t2")
    nc.vector.tensor_mul(t1[:], p4, cb)
    nc.vector.tensor_mul(t2[:], p4s, sb_)
    nc.vector.tensor_add(t1[:], t1[:], t2[:])
    nc.vector.tensor_mul(nat_tile[:].rearrange("p (h x f) -> p h x f", h=nheads, x=2),
                         t1[:], rb)


def build_program():
    cos_np, sin_np = _rope_tables()
    masks_np = _tri_masks()
    # one-hot column matrices for batched sums: oh8[p, h, c] = (c == h)
    oh_col = np.zeros((128, HLOC, HLOC), dtype=np.float32)
    for h in range(HLOC):
        oh_col[:, h, h] = 1.0
    # one-hot row matrices for broadcast: ohr[p, h, c] = (p == h)
    oh_row = np.zeros((HLOC, HLOC, 128), dtype=np.float32)
    for h in range(HLOC):
        oh_row[h, h, :] = 1.0

    nc = bacc.Bacc(trn_type="TRN2")

    xt_d = nc.dram_tensor("xt", [D, S], bf16, kind="ExternalInput")
    wq_d = nc.dram_tensor("wq", [D, HLOC * HD], bf16, kind="ExternalInput")
    wkv_d = nc.dram_tensor("wkv", [D, 2 * KVLOC * HD], bf16, kind="ExternalInput")
    wo_d = nc.dram_tensor("wo", [HLOC * HD, D], bf16, kind="ExternalInput")
    out_d = nc.dram_tensor("out", [S, D], bf16, kind="ExternalOutput")

    cos_d = nc.inline_tensor(cos_np.astype(BF), "cos_t")
    sin_d = nc.inline_tensor(sin_np.astype(BF), "sin_t")
    ident_d = nc.inline_tensor(np.eye(128, dtype=np.float32).astype(BF), "ident")
    masks_d = nc.inline_tensor(masks_np.astype(BF), "tri_masks")
    ohc_d = nc.inline_tensor(oh_col.astype(BF), "oh_col")
    ohr_d = nc.inline_tensor(oh_row.astype(BF), "oh_row")

    with tile.TileContext(nc) as tc:
        with tc.tile_pool(name="cst", bufs=1) as cst:
            cos_sb = cst.tile([128, NTC, 128], bf16, tag="cos")
            sin_sb = cst.tile([128, NTC, 128], bf16, tag="sin")
            ident = cst.tile([128, 128], bf16, tag="ident")
            masks = cst.tile([128, 128], bf16, tag="masks")
            ohc = cst.tile([128, HLOC, HLOC], bf16, tag="ohc")
            ohr = cst.tile([HLOC, HLOC, 128], bf16, tag="ohr")
            eps_sb = cst.tile([128, 1], f32, tag="eps")
            nc.sync.dma_start(out=cos_sb[:], in_=cos_d[:].rearrange("(t p) f -> p t f", p=128))
            nc.sync.dma_start(out=sin_sb[:], in_=sin_d[:].rearrange("(t p) f -> p t f", p=128))
            nc.gpsimd.dma_start(out=ident[:], in_=ident_d[:])
            nc.gpsimd.dma_start(out=masks[:], in_=masks_d[:])
            nc.gpsimd.dma_start(out=ohc[:], in_=ohc_d[:])
            nc.gpsimd.dma_start(out=ohr[:], in_=ohr_d[:])
            nc.gpsimd.memset(eps_sb[:], RMS_EPS)

            # resident tensors
            qt_sb = cst.tile([128, HLOC, S], bf16, tag="qt")
            kt_sb = cst.tile([128, KVLOC, S], bf16, tag="kt")
            v_sb = cst.tile([128, NTC, KVLOC * HD], bf16, tag="v")
            ytn = cst.tile([128, HLOC, S], bf16, tag="ytn")
            wq_sb = cst.tile([128, NDT, HLOC * HD], bf16, tag="wq")
            wkv_sb = cst.tile([128, NDT, 512], bf16, tag="wkv")
            wo_sb = cst.tile([128, HLOC, D], bf16, tag="wo")

            # weight preloads: per-slice, spread across idle queues so the
            # first matmuls can start as soon as their slices land
            wkv_r = wkv_d[:].rearrange("(t p) c -> p t c", p=128)
            wq_r = wq_d[:].rearrange("(t p) c -> p t c", p=128)
            wo_r = wo_d[:].rearrange("(h p) c -> p h c", p=128)
            for dt in range(NDT):
                nc.sync.dma_start(out=wkv_sb[:, dt, :], in_=wkv_r[:, dt, :])
                nc.scalar.dma_start(out=wq_sb[:, dt, :], in_=wq_r[:, dt, :])
            for h in range(HLOC):
                nc.gpsimd.dma_start(out=wo_sb[:, h, :], in_=wo_r[:, h, :])

            # ---- fused per-window pipeline ----
            # for each 512-token window w: P1 (qkv+rms+rope+transpose for its
            # 4 token tiles) -> P2 attention over windows's queries -> batched
            # softmax normalization -> P3 (output projection) for window w-1,
            # interleaved into P2 of the NEXT window as PE gap filler.
            with tc.tile_pool(name="xs", bufs=2) as xs, \
                 tc.tile_pool(name="nat", bufs=2) as nat, \
                 tc.tile_pool(name="ex", bufs=6) as ex, \
                 tc.tile_pool(name="sm", bufs=2) as sm, \
                 tc.tile_pool(name="ob", bufs=2) as ob, \
                 tc.tile_pool(name="acc", bufs=2, space="PSUM") as acc, \
                 tc.tile_pool(name="psc", bufs=3, space="PSUM") as psc, \
                 tc.tile_pool(name="py", bufs=2, space="PSUM") as py, \
                 tc.tile_pool(name="pn", bufs=1, space="PSUM") as pn:

                def emit_p1_group(ps, nheads, cos1, sin1, heads):
                    # RMS+RoPE on psum group, then PE-transpose each head tile
                    # into its resident [d, tok] slot. heads: list of
                    # (dst_tile, dst_head, col0, tcid)
                    qn = nat.tile([128, nheads * 128], bf16, tag="qn")
                    _emit_rms_rope(nc, nat, ps, nheads, cos1, sin1, qn, eps_sb[:])
                    for idx, (dst, dh, c0, tcid) in enumerate(heads):
                        tp = psc.tile([128, 128], bf16, tag="sc")
                        nc.tensor.transpose(tp[:], qn[:, c0:c0 + 128], ident[:])
                        if idx % 2 == 0:
                            nc.vector.tensor_copy(dst[:, dh, tcid * 128:(tcid + 1) * 128], tp[:])
                        else:
                            nc.scalar.activation(dst[:, dh, tcid * 128:(tcid + 1) * 128], tp[:],
                                                 mybir.ActivationFunctionType.Copy)

                def emit_p1_tc(tcid):
                    xt_sb = xs.tile([128, NDT, 128], bf16, tag="xt")
                    nc.sync.dma_start(
                        out=xt_sb[:],
                        in_=xt_d[:, tcid * 128:(tcid + 1) * 128]
                            .rearrange("(t p) s -> p t s", p=128))
                    cos1 = cos_sb[:, tcid:tcid + 1, :]
                    sin1 = sin_sb[:, tcid:tcid + 1, :]
                    # q heads group 1, group 2, then kv -- each group finishes
                    # (rms/rope/transpose emitted) before the next so two
                    # accumulator banks suffice
                    for gi in range(2):
                        ps_q = acc.tile([128, 512], f32, tag="acc")
                        for dt in range(NDT):
                            nc.tensor.matmul(ps_q[:], xt_sb[:, dt, :],
                                             wq_sb[:, dt, gi * 512:(gi + 1) * 512],
                                             start=dt == 0, stop=dt == NDT - 1)
                        emit_p1_group(ps_q[:], 4, cos1, sin1,
                                      [(qt_sb, gi * 4 + hh, hh * 128, tcid) for hh in range(4)])
                    ps_kv = acc.tile([128, 512], f32, tag="acc")
                    for dt in range(NDT):
                        nc.tensor.matmul(ps_kv[:], xt_sb[:, dt, :], wkv_sb[:, dt, :],
                                         start=dt == 0, stop=dt == NDT - 1)
                    emit_p1_group(ps_kv[:, 0:256], 2, cos1, sin1,
                                  [(kt_sb, kh, kh * 128, tcid) for kh in range(KVLOC)])
                    nc.vector.tensor_copy(v_sb[:, tcid, :], ps_kv[:, 256:512])

                def emit_p3_tile(og, tcid):
                    ps_o = acc.tile([128, 512], f32, tag="acc")
                    for h in range(HLOC):
                        nc.tensor.matmul(
                            ps_o[:],
                            ytn[:, h, tcid * 128:(tcid + 1) * 128],
                            wo_sb[:, h, og * 512:(og + 1) * 512],
                            start=(h == 0), stop=(h == HLOC - 1))
                    ot = ob.tile([128, 512], bf16, tag="ot")
                    nc.vector.tensor_copy(ot[:], ps_o[:])
                    nc.gpsimd.dma_start(
                        out=out_d[tcid * 128:(tcid + 1) * 128, og * 512:(og + 1) * 512],
                        in_=ot[:])

                for w in range(NWIN):
                    for tcid in range(4 * w, 4 * w + 4):
                        emit_p1_tc(tcid)

                    # ---- P2 window w (+ P3 of window w-1 as gap filler) ----
                    njt = 4 * w + 4
                    ps_sums = pn.tile([HLOC, 512], f32, tag="sums")
                    for hp in range(HLOC // 2):
                        h0, h1 = 2 * hp, 2 * hp + 1
                        kvh = h0 // 4
                        ps_y0 = py.tile([128, 512], f32, tag="y")
                        ps_y1 = py.tile([128, 512], f32, tag="y")
                        for j in range(njt):
                            vi = j - 4 * w
                            c0 = 128 * vi if vi >= 0 else 0
                            kt_j = kt_sb[:, kvh, j * 128:(j + 1) * 128]
                            v_j = v_sb[:, j, kvh * 128:(kvh + 1) * 128]
                            st, sp = j == 0, j == njt - 1
                            for hq, ps_y in ((h0, ps_y0), (h1, ps_y1)):
                                ps_sc = psc.tile([128, 512], f32, tag="sc")
                                nc.tensor.matmul(
                                    ps_sc[:, c0:512], kt_j,
                                    qt_sb[:, hq, w * 512 + c0:(w + 1) * 512])
                                if vi >= 0:
                                    nc.vector.tensor_add(ps_sc[:, c0:c0 + 128],
                                                         ps_sc[:, c0:c0 + 128],
                                                         masks[:])
                                et = ex.tile([128, 512], bf16, tag="et")
                                nc.scalar.activation(et[:, c0:512], ps_sc[:, c0:512],
                                                     mybir.ActivationFunctionType.Exp,
                                                     scale=SCALE)
                                nc.tensor.matmul(
                                    ps_y[:, c0:512], v_j,
                                    et[:, c0:512], start=st, stop=sp,
                                    skip_group_check=True)
                                nc.tensor.matmul(
                                    ps_sums[:, c0:512], ohc[:, hq, :], et[:, c0:512],
                                    start=(hq == 0 and j == 0),
                                    stop=(hq == HLOC - 1 and j == njt - 1),
                                    skip_group_check=True)
                        nc.vector.tensor_copy(ytn[:, h0, w * 512:(w + 1) * 512], ps_y0[:])
                        nc.vector.tensor_copy(ytn[:, h1, w * 512:(w + 1) * 512], ps_y1[:])
                        # P3 gap filler: one og-stripe of the previous window
                        if w > 0:
                            for tcl in range(4):
                                emit_p3_tile(hp, 4 * (w - 1) + tcl)

                    # batched 1/sums for all 8 heads of this window
                    lgs = sm.tile([HLOC, 512], f32, tag="lgs")
                    nc.scalar.activation(lgs[:], ps_sums[:],
                                         mybir.ActivationFunctionType.Ln)
                    rec = sm.tile([HLOC, 512], bf16, tag="rec")
                    nc.scalar.activation(rec[:], lgs[:],
                                         mybir.ActivationFunctionType.Exp,
                                         scale=-1.0)
                    for hq in range(HLOC):
                        bcp = psc.tile([128, 512], f32, tag="sc")
                        nc.tensor.matmul(bcp[:], ohr[:, hq, :], rec[:]) 
                        nc.vector.tensor_mul(
                            ytn[:, hq, w * 512:(w + 1) * 512],
                            ytn[:, hq, w * 512:(w + 1) * 512], bcp[:])

                # final P3 stripe: window 3
                for og in range(4):
                    for tcl in range(4):
                        emit_p3_tile(og, 12 + tcl)

    nc.compile()
    return nc


_PROGRAM = None


def _get_program():
    global _PROGRAM
    if _PROGRAM is None:
        _PROGRAM = build_program()
    return _PROGRAM


def make_in_maps(x, W_qkv, W_out):
    in_maps = []
    for c in range(8):
        b, t = c // 2, c % 2
        xt = np.ascontiguousarray(x[b].T).astype(BF)
        wq = np.ascontiguousarray(W_qkv[:, t * 1024:(t + 1) * 1024]).astype(BF)
        wk = W_qkv[:, D + t * 256: D + (t + 1) * 256]
        wv = W_qkv[:, D + 512 + t * 256: D + 512 + (t + 1) * 256]
        wkv = np.ascontiguousarray(np.concatenate([wk, wv], axis=1)).astype(BF)
        wo = np.ascontiguousarray(W_out[t * 1024:(t + 1) * 1024, :]).astype(BF)
        in_maps.append({"xt": xt, "wq": wq, "wkv": wkv, "wo": wo})
    return in_maps


def kernel(x, W_qkv, W_out):
    from concourse.bass_utils import run_bass_kernel_spmd
    nc = _get_program()
    in_maps = make_in_maps(np.asarray(x, dtype=np.float32),
                           np.asarray(W_qkv, dtype=np.float32),
                           np.asarray(W_out, dtype=np.float32))
    res = run_bass_kernel_spmd(nc, in_maps, list(range(8)), trace=False)
    out = np.empty((B, S, D), dtype=np.float32)
    for b in range(B):
        out[b] = (res.results[2 * b]["out"].astype(np.float32)
                  + res.results[2 * b + 1]["out"].astype(np.float32))
    return out


# revision 9
# speedup vs baseline: 1.6205x; 1.0065x over previous
"""Causal self-attention (GQA + RMS-norm + RoPE) Trainium2 Bass kernel.

Sharding: 8 cores = 4 batches x 2 head-groups (tensor-parallel over heads).
Core c = 2*b + t handles batch b with Q heads [8t, 8t+8) and KV heads
[2t, 2t+2). Each core computes a partial output projection (its heads'
rows of W_out); the host sums the two partials per batch.

All matmul operands are bf16 (fp32 PSUM accumulation); elementwise /
normalization math stays fp32. Everything (qT, kT, v, ytn) is SBUF
resident -- no DRAM scratch round-trip.

Pipeline per core:
  P1: qkv = x @ W_shard (transposed-x input), RMS+RoPE on q/k in natural
      layout, PE-transpose q/k to [d, tok] into resident SBUF.
  P2: per 512-token query window, per head: scoresT = kT_tile.T @ qT_win,
      +tri-mask on diagonal tiles, exp (ACT, scale=hd^-0.5), then
      yT += v_tile.T @ expT; per-window sums for all 8 heads accumulate
      into one [8,512] PSUM bank via one-hot stationary columns; a single
      Ln/Exp pair per window gives 1/sums, broadcast via one-hot-row
      matmuls, applied on DVE.
  P3: out = sum_h yT_norm_h.T @ W_out_h rows -> partial [S, D] (bf16).

Activation-table note: all ACT functions used (Square/Ln/Exp/Copy) live
in the 'natural_log_exp_and_others' table; we reorder the table list fed
to the act-table-load pass so that table is preferred (and remap the
emitted ids back to act_info.json order), avoiding per-activation table
reloads.
"""
import sys, os
sys.path.insert(0, '/opt/trn_rl_repo')
import numpy as np
import ml_dtypes

import concourse.bacc as bacc_mod
from concourse import bass, bacc, mybir, tile

f32 = mybir.dt.float32
bf16 = mybir.dt.bfloat16
BF = ml_dtypes.bfloat16

B, S, D = 4, 2048, 2048
H, HKV, HD = 16, 4, 128
HLOC = H // 2          # 8 q heads per core
KVLOC = HKV // 2       # 2 kv heads per core
SCALE = float(HD) ** -0.5
RMS_EPS = float(np.finfo(np.float32).eps)
ROPE_BASE = 10000.0

NTC = S // 128         # 16 token tiles
NDT = D // 128         # 16 contraction tiles
NWIN = S // 512        # 4 query windows


# ---- activation-table preference patch: prefer the table holding
# square+ln+exp+copy so the kernel needs a single ACT_TABLE_LOAD. ----
_PREF_TABLE = 'natural_log_exp_and_others'
_orig_insert_atl = bacc_mod._bass_rust.insert_act_table_loads


def _patched_insert_atl(bobj, tables):
    names = [t[0] for t in tables]
    if _PREF_TABLE in names:
        pi = names.index(_PREF_TABLE)
        order = [pi] + [i for i in range(len(tables)) if i != pi]
        _orig_insert_atl(bobj, [tables[i] for i in order])
        for blk in bobj.main_func.blocks:
            for inst in blk.instructions:
                if type(inst).__name__ == 'InstLoadActFuncSet':
                    inst.act_func_set_id = order[inst.act_func_set_id]
    else:
        _orig_insert_atl(bobj, tables)


bacc_mod._bass_rust.insert_act_table_loads = _patched_insert_atl


def _rope_tables():
    inv_freq = (1.0 / (ROPE_BASE ** (np.arange(0, HD, 2, dtype=np.float32) / HD))).astype(np.float32)
    freqs = np.arange(S, dtype=np.float32)[:, None] * inv_freq[None, :]
    cos = np.cos(freqs).astype(np.float32)
    sin = np.sin(freqs).astype(np.float32)
    cos2 = np.concatenate([cos, cos], axis=1)        # [S, 128]
    sin2 = np.concatenate([sin, -sin], axis=1)       # [S, 128]
    return cos2, sin2


def _tri_masks():
    # triangular mask for the 128-col diagonal block of a diag score tile:
    # local col f, row p: masked iff p > f
    m = np.zeros((128, 128), dtype=np.float32)
    p = np.arange(128)[:, None]
    f = np.arange(128)[None, :]
    m[p > f] = -1e30
    return m


def _emit_rms_rope(nc, scr, psum_ap, nheads, cos1, sin1, nat_tile, eps_ap):
    """psum_ap: [128, nheads*128] qkv psum slice; writes RMS+RoPE result into
    nat_tile (SBUF, bf16). cos1/sin1: [128, 1, 128] APs (cos dup, [sin,-sin]).

    rot(q) = q*cos2 + swap_halves(q)*sin2;  out = rot(q) * rsqrt(mean(q^2)+eps)
    rsqrt computed as exp(-0.5*ln(ss/128+eps)) on ACT.
    """
    w = nheads * 128
    sq = scr.tile([128, w], bf16, tag="sq")
    nc.scalar.activation(sq[:], psum_ap, mybir.ActivationFunctionType.Square)
    ss = scr.tile([128, nheads, 1], f32, tag="ss")
    nc.vector.tensor_reduce(
        ss[:], sq[:].rearrange("p (h f) -> p h f", h=nheads),
        axis=mybir.AxisListType.X, op=mybir.AluOpType.add)
    lg = scr.tile([128, nheads, 1], f32, tag="lg")
    nc.scalar.activation(lg[:], ss[:], mybir.ActivationFunctionType.Ln,
                         scale=1.0 / HD, bias=eps_ap)
    rinv = scr.tile([128, nheads, 1, 1], f32, tag="rinv")
    nc.scalar.activation(rinv[:], lg[:], mybir.ActivationFunctionType.Exp,
                         scale=-0.5)

    shp = [128, nheads, 2, 64]
    p4 = psum_ap.rearrange("p (h x f) -> p h x f", h=nheads, x=2)
    p4s = p4[:, :, ::-1, :]
    cb = cos1.rearrange("p t (x f) -> p t x f", x=2).to_broadcast(shp)
    sb_ = sin1.rearrange("p t (x f) -> p t x f", x=2).to_broadcast(shp)
    rb = rinv[:].to_broadcast(shp)
    t1 = scr.tile(shp, bf16, tag="t1")
    t2 = scr.tile(shp, bf16, tag="t2")
    nc.vector.tensor_mul(t1[:], p4, cb)
    nc.vector.tensor_mul(t2[:], p4s, sb_)
    nc.vector.tensor_add(t1[:], t1[:], t2[:])
    nc.vector.tensor_mul(nat_tile[:].rearrange("p (h x f) -> p h x f", h=nheads, x=2),
                         t1[:], rb)


def build_program():
    cos_np, sin_np = _rope_tables()
    masks_np = _tri_masks()
    # one-hot column matrices for batched sums: oh8[p, h, c] = (c == h)
    oh_col = np.zeros((128, HLOC, HLOC), dtype=np.float32)
    for h in range(HLOC):
        oh_col[:, h, h] = 1.0
    # one-hot row matrices for broadcast: ohr[p, h, c] = (p == h)
    oh_row = np.zeros((HLOC, HLOC, 128), dtype=np.float32)
    for h in range(HLOC):
        oh_row[h, h, :] = 1.0

    nc = bacc.Bacc(trn_type="TRN2")

    xt_d = nc.dram_tensor("xt", [D, S], bf16, kind="ExternalInput")
    wq_d = nc.dram_tensor("wq", [D, HLOC * HD], bf16, kind="ExternalInput")
    wkv_d = nc.dram_tensor("wkv", [D, 2 * KVLOC * HD], bf16, kind="ExternalInput")
    wo_d = nc.dram_tensor("wo", [HLOC * HD, D], bf16, kind="ExternalInput")
    out_d = nc.dram_tensor("out", [S, D], bf16, kind="ExternalOutput")

    cos_d = nc.inline_tensor(cos_np.astype(BF), "cos_t")
    sin_d = nc.inline_tensor(sin_np.astype(BF), "sin_t")
    ident_d = nc.inline_tensor(np.eye(128, dtype=np.float32).astype(BF), "ident")
    masks_d = nc.inline_tensor(masks_np.astype(BF), "tri_masks")
    ohc_d = nc.inline_tensor(oh_col.astype(BF), "oh_col")
    ohr_d = nc.inline_tensor(oh_row.astype(BF), "oh_row")

    with tile.TileContext(nc) as tc:
        with tc.tile_pool(name="cst", bufs=1) as cst:
            cos_sb = cst.tile([128, NTC, 128], bf16, tag="cos")
            sin_sb = cst.tile([128, NTC, 128], bf16, tag="sin")
            ident = cst.tile([128, 128], bf16, tag="ident")
            masks = cst.tile([128, 128], bf16, tag="masks")
            ohc = cst.tile([128, HLOC, HLOC], bf16, tag="ohc")
            ohr = cst.tile([HLOC, HLOC, 128], bf16, tag="ohr")
            eps_sb = cst.tile([128, 1], f32, tag="eps")

            nc.gpsimd.dma_start(out=ident[:], in_=ident_d[:])
            nc.gpsimd.dma_start(out=masks[:], in_=masks_d[:])
            nc.gpsimd.dma_start(out=ohc[:], in_=ohc_d[:])
            nc.gpsimd.dma_start(out=ohr[:], in_=ohr_d[:])
            nc.gpsimd.memset(eps_sb[:], RMS_EPS)

            # resident tensors
            qt_sb = cst.tile([128, HLOC, S], bf16, tag="qt")
            kt_sb = cst.tile([128, KVLOC, S], bf16, tag="kt")
            v_sb = cst.tile([128, NTC, KVLOC * HD], bf16, tag="v")
            ytn = cst.tile([128, HLOC, S], bf16, tag="ytn")
            wq_sb = cst.tile([128, NDT, HLOC * HD], bf16, tag="wq")
            wkv_sb = cst.tile([128, NDT, 512], bf16, tag="wkv")
            wo_sb = cst.tile([128, HLOC, D], bf16, tag="wo")

            # weight preloads: per-slice, spread across idle queues so the
            # first matmuls can start as soon as their slices land
            wkv_r = wkv_d[:].rearrange("(t p) c -> p t c", p=128)
            wq_r = wq_d[:].rearrange("(t p) c -> p t c", p=128)
            wo_r = wo_d[:].rearrange("(h p) c -> p h c", p=128)
            for dt in range(NDT):
                nc.gpsimd.dma_start(out=wkv_sb[:, dt, :], in_=wkv_r[:, dt, :])
                nc.scalar.dma_start(out=wq_sb[:, dt, :], in_=wq_r[:, dt, :])
            for h in range(HLOC):
                nc.gpsimd.dma_start(out=wo_sb[:, h, :], in_=wo_r[:, h, :])

            # ---- fused per-window pipeline ----
            # for each 512-token window w: P1 (qkv+rms+rope+transpose for its
            # 4 token tiles) -> P2 attention over windows's queries -> batched
            # softmax normalization -> P3 (output projection) for window w-1,
            # interleaved into P2 of the NEXT window as PE gap filler.
            with tc.tile_pool(name="xs", bufs=2) as xs, \
                 tc.tile_pool(name="nat", bufs=2) as nat, \
                 tc.tile_pool(name="ex", bufs=8) as ex, \
                 tc.tile_pool(name="sm", bufs=2) as sm, \
                 tc.tile_pool(name="ob", bufs=2) as ob, \
                 tc.tile_pool(name="acc", bufs=2, space="PSUM") as acc, \
                 tc.tile_pool(name="psc", bufs=3, space="PSUM") as psc, \
                 tc.tile_pool(name="py", bufs=2, space="PSUM") as py, \
                 tc.tile_pool(name="pn", bufs=1, space="PSUM") as pn:

                # prefetch x for the first two token tiles ahead of the
                # constant/weight loads so the first matmuls start immediately
                xt_pre = {}
                for tcid in (0, 1):
                    xt_sb = xs.tile([128, NDT, 128], bf16, tag="xt")
                    nc.sync.dma_start(
                        out=xt_sb[:],
                        in_=xt_d[:, tcid * 128:(tcid + 1) * 128]
                            .rearrange("(t p) s -> p t s", p=128))
                    xt_pre[tcid] = xt_sb
                nc.sync.dma_start(out=cos_sb[:], in_=cos_d[:].rearrange("(t p) f -> p t f", p=128))
                nc.sync.dma_start(out=sin_sb[:], in_=sin_d[:].rearrange("(t p) f -> p t f", p=128))

                def emit_p1_group(ps, nheads, cos1, sin1, heads):
                    # RMS+RoPE on psum group, then PE-transpose each head tile
                    # into its resident [d, tok] slot. heads: list of
                    # (dst_tile, dst_head, col0, tcid)
                    qn = nat.tile([128, nheads * 128], bf16, tag="qn")
                    _emit_rms_rope(nc, nat, ps, nheads, cos1, sin1, qn, eps_sb[:])
                    for idx, (dst, dh, c0, tcid) in enumerate(heads):
                        tp = psc.tile([128, 128], bf16, tag="sc")
                        nc.tensor.transpose(tp[:], qn[:, c0:c0 + 128], ident[:])
                        if idx % 2 == 0:
                            nc.vector.tensor_copy(dst[:, dh, tcid * 128:(tcid + 1) * 128], tp[:])
                        else:
                            nc.scalar.activation(dst[:, dh, tcid * 128:(tcid + 1) * 128], tp[:],
                                                 mybir.ActivationFunctionType.Copy)

                def emit_p1_tc(tcid):
                    if tcid in xt_pre:
                        xt_sb = xt_pre.pop(tcid)
                    else:
                        xt_sb = xs.tile([128, NDT, 128], bf16, tag="xt")
                        nc.sync.dma_start(
                            out=xt_sb[:],
                            in_=xt_d[:, tcid * 128:(tcid + 1) * 128]
                                .rearrange("(t p) s -> p t s", p=128))
                    cos1 = cos_sb[:, tcid:tcid + 1, :]
                    sin1 = sin_sb[:, tcid:tcid + 1, :]
                    # q heads group 1, group 2, then kv -- each group finishes
                    # (rms/rope/transpose emitted) before the next so two
                    # accumulator banks suffice
                    for gi in range(2):
                        ps_q = acc.tile([128, 512], f32, tag="acc")
                        for dt in range(NDT):
                            nc.tensor.matmul(ps_q[:], xt_sb[:, dt, :],
                                             wq_sb[:, dt, gi * 512:(gi + 1) * 512],
                                             start=dt == 0, stop=dt == NDT - 1)
                        emit_p1_group(ps_q[:], 4, cos1, sin1,
                                      [(qt_sb, gi * 4 + hh, hh * 128, tcid) for hh in range(4)])
                    ps_kv = acc.tile([128, 512], f32, tag="acc")
                    for dt in range(NDT):
                        nc.tensor.matmul(ps_kv[:], xt_sb[:, dt, :], wkv_sb[:, dt, :],
                                         start=dt == 0, stop=dt == NDT - 1)
                    emit_p1_group(ps_kv[:, 0:256], 2, cos1, sin1,
                                  [(kt_sb, kh, kh * 128, tcid) for kh in range(KVLOC)])
                    nc.vector.tensor_copy(v_sb[:, tcid, :], ps_kv[:, 256:512])

                def emit_p3_tile(og, tcid):
                    ps_o = acc.tile([128, 512], f32, tag="acc")
                    for h in range(HLOC):
                        nc.tensor.matmul(
                            ps_o[:],
                            ytn[:, h, tcid * 128:(tcid + 1) * 128],
                            wo_sb[:, h, og * 512:(og + 1) * 512],
                            start=(h == 0), stop=(h == HLOC - 1))
                    ot = ob.tile([128, 512], bf16, tag="ot")
                    nc.vector.tensor_copy(ot[:], ps_o[:])
                    nc.gpsimd.dma_start(
                        out=out_d[tcid * 128:(tcid + 1) * 128, og * 512:(og + 1) * 512],
                        in_=ot[:])

                for w in range(NWIN):
                    for tcid in range(4 * w, 4 * w + 4):
                        emit_p1_tc(tcid)

                    # ---- P2 window w (+ P3 of window w-1 as gap filler) ----
                    njt = 4 * w + 4
                    ps_sums = pn.tile([HLOC, 512], f32, tag="sums")
                    for hp in range(HLOC // 2):
                        h0, h1 = 2 * hp, 2 * hp + 1
                        kvh = h0 // 4
                        ps_y0 = py.tile([128, 512], f32, tag="y")
                        ps_y1 = py.tile([128, 512], f32, tag="y")
                        for j in range(njt):
                            vi = j - 4 * w
                            c0 = 128 * vi if vi >= 0 else 0
                            kt_j = kt_sb[:, kvh, j * 128:(j + 1) * 128]
                            v_j = v_sb[:, j, kvh * 128:(kvh + 1) * 128]
                            st, sp = j == 0, j == njt - 1
                            for hq, ps_y in ((h0, ps_y0), (h1, ps_y1)):
                                ps_sc = psc.tile([128, 512], f32, tag="sc")
                                nc.tensor.matmul(
                                    ps_sc[:, c0:512], kt_j,
                                    qt_sb[:, hq, w * 512 + c0:(w + 1) * 512])
                                if vi >= 0:
                                    nc.vector.tensor_add(ps_sc[:, c0:c0 + 128],
                                                         ps_sc[:, c0:c0 + 128],
                                                         masks[:])
                                et = ex.tile([128, 512], bf16, tag="et")
                                nc.scalar.activation(et[:, c0:512], ps_sc[:, c0:512],
                                                     mybir.ActivationFunctionType.Exp,
                                                     scale=SCALE)
                                nc.tensor.matmul(
                                    ps_y[:, c0:512], v_j,
                                    et[:, c0:512], start=st, stop=sp,
                                    skip_group_check=True)
                                nc.tensor.matmul(
                                    ps_sums[:, c0:512], ohc[:, hq, :], et[:, c0:512],
                                    start=(hq == 0 and j == 0),
                                    stop=(hq == HLOC - 1 and j == njt - 1),
                                    skip_group_check=True)
                        nc.vector.tensor_copy(ytn[:, h0, w * 512:(w + 1) * 512], ps_y0[:])
                        nc.vector.tensor_copy(ytn[:, h1, w * 512:(w + 1) * 512], ps_y1[:])
                        # P3 gap filler: one og-stripe of the previous window
                        if w > 0:
                            for tcl in range(4):
                                emit_p3_tile(hp, 4 * (w - 1) + tcl)

                    # batched 1/sums for all 8 heads of this window
                    lgs = sm.tile([HLOC, 512], f32, tag="lgs")
                    nc.scalar.activation(lgs[:], ps_sums[:],
                                         mybir.ActivationFunctionType.Ln)
                    rec = sm.tile([HLOC, 512], bf16, tag="rec")
                    nc.scalar.activation(rec[:], lgs[:],
                                         mybir.ActivationFunctionType.Exp,
                                         scale=-1.0)
                    for hq in range(HLOC):
                        bcp = psc.tile([128, 512], f32, tag="sc")
                        nc.tensor.matmul(bcp[:], ohr[:, hq, :], rec[:]) 
                        nc.vector.tensor_mul(
                            ytn[:, hq, w * 512:(w + 1) * 512],
                            ytn[:, hq, w * 512:(w + 1) * 512], bcp[:])

                # final P3 stripe: window 3
                for og in range(4):
                    for tcl in range(4):
                        emit_p3_tile(og, 12 + tcl)

    nc.compile()
    return nc


_PROGRAM = None


def _get_program():
    global _PROGRAM
    if _PROGRAM is None:
        _PROGRAM = build_program()
    return _PROGRAM


def make_in_maps(x, W_qkv, W_out):
    in_maps = []
    for c in range(8):
        b, t = c // 2, c % 2
        xt = np.ascontiguousarray(x[b].T).astype(BF)
        wq = np.ascontiguousarray(W_qkv[:, t * 1024:(t + 1) * 1024]).astype(BF)
        wk = W_qkv[:, D + t * 256: D + (t + 1) * 256]
        wv = W_qkv[:, D + 512 + t * 256: D + 512 + (t + 1) * 256]
        wkv = np.ascontiguousarray(np.concatenate([wk, wv], axis=1)).astype(BF)
        wo = np.ascontiguousarray(W_out[t * 1024:(t + 1) * 1024, :]).astype(BF)
        in_maps.append({"xt": xt, "wq": wq, "wkv": wkv, "wo": wo})
    return in_maps


def kernel(x, W_qkv, W_out):
    from concourse.bass_utils import run_bass_kernel_spmd
    nc = _get_program()
    in_maps = make_in_maps(np.asarray(x, dtype=np.float32),
                           np.asarray(W_qkv, dtype=np.float32),
                           np.asarray(W_out, dtype=np.float32))
    res = run_bass_kernel_spmd(nc, in_maps, list(range(8)), trace=False)
    out = np.empty((B, S, D), dtype=np.float32)
    for b in range(B):
        out[b] = (res.results[2 * b]["out"].astype(np.float32)
                  + res.results[2 * b + 1]["out"].astype(np.float32))
    return out


# revision 11
# speedup vs baseline: 1.7791x; 1.0979x over previous
"""Causal self-attention (GQA + RMS-norm + RoPE) Trainium2 Bass kernel.

Sharding: 8 cores = 4 batches x 2 head-groups (tensor-parallel over heads).
Core c = 2*b + t handles batch b with Q heads [8t, 8t+8) and KV heads
[2t, 2t+2). Each core computes a partial output projection (its heads'
rows of W_out); the host sums the two partials per batch.

All matmul operands are bf16 (fp32 PSUM accumulation); elementwise /
normalization math stays fp32. Everything (qT, kT, v, ytn) is SBUF
resident -- no DRAM scratch round-trip.

Pipeline per core:
  P1: qkv = x @ W_shard (transposed-x input), RMS+RoPE on q/k in natural
      layout, PE-transpose q/k to [d, tok] into resident SBUF.
  P2: per 512-token query window, per head: scoresT = kT_tile.T @ qT_win,
      +tri-mask on diagonal tiles, exp (ACT, scale=hd^-0.5), then
      yT += v_tile.T @ expT; per-window sums for all 8 heads accumulate
      into one [8,512] PSUM bank via one-hot stationary columns; a single
      Ln/Exp pair per window gives 1/sums, broadcast via one-hot-row
      matmuls, applied on DVE.
  P3: out = sum_h yT_norm_h.T @ W_out_h rows -> partial [S, D] (bf16).

Activation-table note: all ACT functions used (Square/Ln/Exp/Copy) live
in the 'natural_log_exp_and_others' table; we reorder the table list fed
to the act-table-load pass so that table is preferred (and remap the
emitted ids back to act_info.json order), avoiding per-activation table
reloads.
"""
import sys, os
sys.path.insert(0, '/opt/trn_rl_repo')
import numpy as np
import ml_dtypes

import concourse.bacc as bacc_mod
from concourse import bass, bacc, mybir, tile

f32 = mybir.dt.float32
bf16 = mybir.dt.bfloat16
BF = ml_dtypes.bfloat16

B, S, D = 4, 2048, 2048
H, HKV, HD = 16, 4, 128
HLOC = H // 2          # 8 q heads per core
KVLOC = HKV // 2       # 2 kv heads per core
SCALE = float(HD) ** -0.5
RMS_EPS = float(np.finfo(np.float32).eps)
ROPE_BASE = 10000.0

NTC = S // 128         # 16 token tiles
NDT = D // 128         # 16 contraction tiles
NWIN = S // 512        # 4 query windows


# ---- activation-table preference patch: prefer the table holding
# square+ln+exp+copy so the kernel needs a single ACT_TABLE_LOAD. ----
_PREF_TABLE = 'natural_log_exp_and_others'
_orig_insert_atl = bacc_mod._bass_rust.insert_act_table_loads


def _patched_insert_atl(bobj, tables):
    names = [t[0] for t in tables]
    if _PREF_TABLE in names:
        pi = names.index(_PREF_TABLE)
        order = [pi] + [i for i in range(len(tables)) if i != pi]
        _orig_insert_atl(bobj, [tables[i] for i in order])
        for blk in bobj.main_func.blocks:
            for inst in blk.instructions:
                if type(inst).__name__ == 'InstLoadActFuncSet':
                    inst.act_func_set_id = order[inst.act_func_set_id]
    else:
        _orig_insert_atl(bobj, tables)


bacc_mod._bass_rust.insert_act_table_loads = _patched_insert_atl


def _rope_tables():
    inv_freq = (1.0 / (ROPE_BASE ** (np.arange(0, HD, 2, dtype=np.float32) / HD))).astype(np.float32)
    freqs = np.arange(S, dtype=np.float32)[:, None] * inv_freq[None, :]
    cos = np.cos(freqs).astype(np.float32)
    sin = np.sin(freqs).astype(np.float32)
    cos2 = np.concatenate([cos, cos], axis=1)        # [S, 128]
    sin2 = np.concatenate([sin, -sin], axis=1)       # [S, 128]
    return cos2, sin2


def _tri_masks():
    # triangular mask for the 128-col diagonal block of a diag score tile:
    # local col f, row p: masked iff p > f
    m = np.zeros((128, 128), dtype=np.float32)
    p = np.arange(128)[:, None]
    f = np.arange(128)[None, :]
    m[p > f] = -1e30
    return m


def _emit_rms_rope(nc, scr, psum_ap, nheads, cos1, sin1, nat_tile, eps_ap):
    """psum_ap: [128, nheads*128] qkv psum slice; writes RMS+RoPE result into
    nat_tile (SBUF, bf16). cos1/sin1: [128, 1, 128] APs (cos dup, [sin,-sin]).

    rot(q) = q*cos2 + swap_halves(q)*sin2;  out = rot(q) * rsqrt(mean(q^2)+eps)
    rsqrt computed as exp(-0.5*ln(ss/128+eps)) on ACT.
    """
    w = nheads * 128
    sq = scr.tile([128, w], bf16, tag="sq")
    nc.scalar.activation(sq[:], psum_ap, mybir.ActivationFunctionType.Square)
    ss = scr.tile([128, nheads, 1], f32, tag="ss")
    nc.vector.tensor_reduce(
        ss[:], sq[:].rearrange("p (h f) -> p h f", h=nheads),
        axis=mybir.AxisListType.X, op=mybir.AluOpType.add)
    lg = scr.tile([128, nheads, 1], f32, tag="lg")
    nc.scalar.activation(lg[:], ss[:], mybir.ActivationFunctionType.Ln,
                         scale=1.0 / HD, bias=eps_ap)
    rinv = scr.tile([128, nheads, 1, 1], f32, tag="rinv")
    nc.scalar.activation(rinv[:], lg[:], mybir.ActivationFunctionType.Exp,
                         scale=-0.5)

    shp = [128, nheads, 2, 64]
    p4 = psum_ap.rearrange("p (h x f) -> p h x f", h=nheads, x=2)
    p4s = p4[:, :, ::-1, :]
    cb = cos1.rearrange("p t (x f) -> p t x f", x=2).to_broadcast(shp)
    sb_ = sin1.rearrange("p t (x f) -> p t x f", x=2).to_broadcast(shp)
    rb = rinv[:].to_broadcast(shp)
    t1 = scr.tile(shp, bf16, tag="t1")
    t2 = scr.tile(shp, bf16, tag="t2")
    nc.vector.tensor_mul(t1[:], p4, cb)
    nc.vector.tensor_mul(t2[:], p4s, sb_)
    nc.vector.tensor_add(t1[:], t1[:], t2[:])
    nc.vector.tensor_mul(nat_tile[:].rearrange("p (h x f) -> p h x f", h=nheads, x=2),
                         t1[:], rb)


def build_program():
    cos_np, sin_np = _rope_tables()
    masks_np = _tri_masks()
    # one-hot column matrices for batched sums: oh8[p, h, c] = (c == h)
    oh_col = np.zeros((128, HLOC, HLOC), dtype=np.float32)
    for h in range(HLOC):
        oh_col[:, h, h] = 1.0
    # one-hot row matrices for broadcast: ohr[p, h, c] = (p == h)
    oh_row = np.zeros((HLOC, HLOC, 128), dtype=np.float32)
    for h in range(HLOC):
        oh_row[h, h, :] = 1.0

    nc = bacc.Bacc(trn_type="TRN2")

    xt_d = nc.dram_tensor("xt", [D, S], bf16, kind="ExternalInput")
    wq_d = nc.dram_tensor("wq", [D, HLOC * HD], bf16, kind="ExternalInput")
    wkv_d = nc.dram_tensor("wkv", [D, 2 * KVLOC * HD], bf16, kind="ExternalInput")
    wo_d = nc.dram_tensor("wo", [HLOC * HD, D], bf16, kind="ExternalInput")
    out_d = nc.dram_tensor("out", [S, D], bf16, kind="ExternalOutput")

    cos_d = nc.inline_tensor(cos_np.astype(BF), "cos_t")
    sin_d = nc.inline_tensor(sin_np.astype(BF), "sin_t")
    ident_d = nc.inline_tensor(np.eye(128, dtype=np.float32).astype(BF), "ident")
    masks_d = nc.inline_tensor(masks_np.astype(BF), "tri_masks")
    ohc_d = nc.inline_tensor(oh_col.astype(BF), "oh_col")
    ohr_d = nc.inline_tensor(oh_row.astype(BF), "oh_row")

    with tile.TileContext(nc) as tc:
        with tc.tile_pool(name="cst", bufs=1) as cst:
            cos_sb = cst.tile([128, NTC, 128], bf16, tag="cos")
            sin_sb = cst.tile([128, NTC, 128], bf16, tag="sin")
            ident = cst.tile([128, 128], bf16, tag="ident")
            masks = cst.tile([128, 128], bf16, tag="masks")
            ohc = cst.tile([128, HLOC, HLOC], bf16, tag="ohc")
            ohr = cst.tile([HLOC, HLOC, 128], bf16, tag="ohr")
            eps_sb = cst.tile([128, 1], f32, tag="eps")

            nc.gpsimd.dma_start(out=ident[:], in_=ident_d[:])
            nc.gpsimd.dma_start(out=masks[:], in_=masks_d[:])
            nc.gpsimd.dma_start(out=ohc[:], in_=ohc_d[:])
            nc.gpsimd.dma_start(out=ohr[:], in_=ohr_d[:])
            nc.gpsimd.memset(eps_sb[:], RMS_EPS)

            # resident tensors
            qt_sb = cst.tile([128, HLOC, S], bf16, tag="qt")
            kt_sb = cst.tile([128, KVLOC, S], bf16, tag="kt")
            v_sb = cst.tile([128, NTC, KVLOC * HD], bf16, tag="v")
            ytn = cst.tile([128, HLOC, S], bf16, tag="ytn")
            wq_sb = cst.tile([128, NDT, HLOC * HD], bf16, tag="wq")
            wkv_sb = cst.tile([128, NDT, 512], bf16, tag="wkv")
            wo_sb = cst.tile([128, HLOC, D], bf16, tag="wo")

            # weight preloads: per-slice, spread across idle queues so the
            # first matmuls can start as soon as their slices land
            wkv_r = wkv_d[:].rearrange("(t p) c -> p t c", p=128)
            wq_r = wq_d[:].rearrange("(t p) c -> p t c", p=128)
            wo_r = wo_d[:].rearrange("(h p) c -> p h c", p=128)
            for dt in range(NDT):
                nc.gpsimd.dma_start(out=wkv_sb[:, dt, :], in_=wkv_r[:, dt, :])
            for dt in range(0, NDT, 2):
                nc.scalar.dma_start(out=wq_sb[:, dt, :], in_=wq_r[:, dt, :])
            for h in range(HLOC):
                nc.gpsimd.dma_start(out=wo_sb[:, h, :], in_=wo_r[:, h, :])

            # ---- fused per-window pipeline ----
            # for each 512-token window w: P1 (qkv+rms+rope+transpose for its
            # 4 token tiles) -> P2 attention over windows's queries -> batched
            # softmax normalization -> P3 (output projection) for window w-1,
            # interleaved into P2 of the NEXT window as PE gap filler.
            with tc.tile_pool(name="xs", bufs=2) as xs, \
                 tc.tile_pool(name="nat", bufs=2) as nat, \
                 tc.tile_pool(name="ex", bufs=6) as ex, \
                 tc.tile_pool(name="sm", bufs=2) as sm, \
                 tc.tile_pool(name="eu", bufs=3) as eu, \
                 tc.tile_pool(name="ob", bufs=2) as ob, \
                 tc.tile_pool(name="acc", bufs=2, space="PSUM") as acc, \
                 tc.tile_pool(name="psc", bufs=3, space="PSUM") as psc, \
                 tc.tile_pool(name="py", bufs=2, space="PSUM") as py, \
                 tc.tile_pool(name="pn", bufs=1, space="PSUM") as pn:

                # prefetch x for the first two token tiles ahead of the
                # constant/weight loads so the first matmuls start immediately
                xt_pre = {}
                for tcid in (0, 1):
                    xt_sb = xs.tile([128, NDT, 128], bf16, tag="xt")
                    nc.sync.dma_start(
                        out=xt_sb[:],
                        in_=xt_d[:, tcid * 128:(tcid + 1) * 128]
                            .rearrange("(t p) s -> p t s", p=128))
                    xt_pre[tcid] = xt_sb
                nc.sync.dma_start(out=cos_sb[:], in_=cos_d[:].rearrange("(t p) f -> p t f", p=128))
                nc.sync.dma_start(out=sin_sb[:], in_=sin_d[:].rearrange("(t p) f -> p t f", p=128))
                for dt in range(1, NDT, 2):
                    nc.sync.dma_start(out=wq_sb[:, dt, :], in_=wq_r[:, dt, :])

                def emit_p1_group(ps, nheads, cos1, sin1, heads):
                    # RMS+RoPE on psum group, then PE-transpose each head tile
                    # into its resident [d, tok] slot. heads: list of
                    # (dst_tile, dst_head, col0, tcid)
                    qn = nat.tile([128, nheads * 128], bf16, tag="qn")
                    _emit_rms_rope(nc, nat, ps, nheads, cos1, sin1, qn, eps_sb[:])
                    for idx, (dst, dh, c0, tcid) in enumerate(heads):
                        tp = psc.tile([128, 128], bf16, tag="sc")
                        nc.tensor.transpose(tp[:], qn[:, c0:c0 + 128], ident[:])
                        if idx % 2 == 0:
                            nc.vector.tensor_copy(dst[:, dh, tcid * 128:(tcid + 1) * 128], tp[:])
                        else:
                            nc.scalar.activation(dst[:, dh, tcid * 128:(tcid + 1) * 128], tp[:],
                                                 mybir.ActivationFunctionType.Copy)

                def emit_p1_tc(tcid):
                    if tcid in xt_pre:
                        xt_sb = xt_pre.pop(tcid)
                    else:
                        xt_sb = xs.tile([128, NDT, 128], bf16, tag="xt")
                        nc.sync.dma_start(
                            out=xt_sb[:],
                            in_=xt_d[:, tcid * 128:(tcid + 1) * 128]
                                .rearrange("(t p) s -> p t s", p=128))
                    cos1 = cos_sb[:, tcid:tcid + 1, :]
                    sin1 = sin_sb[:, tcid:tcid + 1, :]
                    # q heads group 1, group 2, then kv -- each group finishes
                    # (rms/rope/transpose emitted) before the next so two
                    # accumulator banks suffice
                    for gi in range(2):
                        ps_q = acc.tile([128, 512], f32, tag="acc")
                        for dt in range(NDT):
                            nc.tensor.matmul(ps_q[:], xt_sb[:, dt, :],
                                             wq_sb[:, dt, gi * 512:(gi + 1) * 512],
                                             start=dt == 0, stop=dt == NDT - 1)
                        emit_p1_group(ps_q[:], 4, cos1, sin1,
                                      [(qt_sb, gi * 4 + hh, hh * 128, tcid) for hh in range(4)])
                    ps_kv = acc.tile([128, 512], f32, tag="acc")
                    for dt in range(NDT):
                        nc.tensor.matmul(ps_kv[:], xt_sb[:, dt, :], wkv_sb[:, dt, :],
                                         start=dt == 0, stop=dt == NDT - 1)
                    emit_p1_group(ps_kv[:, 0:256], 2, cos1, sin1,
                                  [(kt_sb, kh, kh * 128, tcid) for kh in range(KVLOC)])
                    nc.vector.tensor_copy(v_sb[:, tcid, :], ps_kv[:, 256:512])

                def emit_p3_tile(og, tcid):
                    ps_o = acc.tile([128, 512], f32, tag="acc")
                    for h in range(HLOC):
                        nc.tensor.matmul(
                            ps_o[:],
                            ytn[:, h, tcid * 128:(tcid + 1) * 128],
                            wo_sb[:, h, og * 512:(og + 1) * 512],
                            start=(h == 0), stop=(h == HLOC - 1))
                    ot = ob.tile([128, 512], bf16, tag="ot")
                    nc.vector.tensor_copy(ot[:], ps_o[:])
                    nc.gpsimd.dma_start(
                        out=out_d[tcid * 128:(tcid + 1) * 128, og * 512:(og + 1) * 512],
                        in_=ot[:])

                for w in range(NWIN):
                    for tcid in range(4 * w, 4 * w + 4):
                        emit_p1_tc(tcid)

                    # ---- P2 window w (+ P3 of window w-1 as gap filler) ----
                    njt = 4 * w + 4
                    ps_sums = pn.tile([HLOC, 512], f32, tag="sums")
                    for hp in range(HLOC // 2):
                        h0, h1 = 2 * hp, 2 * hp + 1
                        kvh = h0 // 4
                        ps_y0 = py.tile([128, 512], f32, tag="y")
                        ps_y1 = py.tile([128, 512], f32, tag="y")
                        et_prev = {h0: None, h1: None}
                        for j in range(njt):
                            vi = j - 4 * w
                            c0 = 128 * vi if vi >= 0 else 0
                            kt_j = kt_sb[:, kvh, j * 128:(j + 1) * 128]
                            v_j = v_sb[:, j, kvh * 128:(kvh + 1) * 128]
                            st, sp = j == 0, j == njt - 1
                            for hq, ps_y in ((h0, ps_y0), (h1, ps_y1)):
                                ps_sc = psc.tile([128, 512], f32, tag="sc")
                                nc.tensor.matmul(
                                    ps_sc[:, c0:512], kt_j,
                                    qt_sb[:, hq, w * 512 + c0:(w + 1) * 512])
                                if vi >= 0:
                                    nc.vector.tensor_add(ps_sc[:, c0:c0 + 128],
                                                         ps_sc[:, c0:c0 + 128],
                                                         masks[:])
                                et = ex.tile([128, 512], bf16, tag="et")
                                nc.scalar.activation(et[:, c0:512], ps_sc[:, c0:512],
                                                     mybir.ActivationFunctionType.Exp,
                                                     scale=SCALE)
                                nc.tensor.matmul(
                                    ps_y[:, c0:512], v_j,
                                    et[:, c0:512], start=st, stop=sp,
                                    skip_group_check=True)
                                # sums: pair adjacent et tiles on DVE, halving
                                # the ones-matmul streaming on PE
                                if j % 2 == 0 and not sp:
                                    et_prev[hq] = (et, c0)
                                else:
                                    if et_prev[hq] is not None:
                                        etp, c0p = et_prev[hq]
                                        au = eu.tile([128, 512], bf16, tag="au")
                                        if c0 > c0p:
                                            nc.vector.tensor_copy(au[:, c0p:c0],
                                                                  etp[:, c0p:c0])
                                        nc.vector.tensor_add(au[:, c0:512],
                                                             etp[:, c0:512],
                                                             et[:, c0:512])
                                        su, cs = au, c0p
                                        et_prev[hq] = None
                                    else:
                                        su, cs = et, c0
                                    nc.tensor.matmul(
                                        ps_sums[:, cs:512], ohc[:, hq, :], su[:, cs:512],
                                        start=(hq == 0 and j <= 1),
                                        stop=(hq == HLOC - 1 and sp),
                                        skip_group_check=True)
                        nc.vector.tensor_copy(ytn[:, h0, w * 512:(w + 1) * 512], ps_y0[:])
                        nc.vector.tensor_copy(ytn[:, h1, w * 512:(w + 1) * 512], ps_y1[:])
                        # P3 gap filler: one og-stripe of the previous window
                        if w > 0:
                            for tcl in range(4):
                                emit_p3_tile(hp, 4 * (w - 1) + tcl)

                    # batched 1/sums for all 8 heads of this window
                    lgs = sm.tile([HLOC, 512], f32, tag="lgs")
                    nc.scalar.activation(lgs[:], ps_sums[:],
                                         mybir.ActivationFunctionType.Ln)
                    rec = sm.tile([HLOC, 512], bf16, tag="rec")
                    nc.scalar.activation(rec[:], lgs[:],
                                         mybir.ActivationFunctionType.Exp,
                                         scale=-1.0)
                    for hq in range(HLOC):
                        bcp = psc.tile([128, 512], f32, tag="sc")
                        nc.tensor.matmul(bcp[:], ohr[:, hq, :], rec[:])
                        nc.vector.tensor_mul(
                            ytn[:, hq, w * 512:(w + 1) * 512],
                            ytn[:, hq, w * 512:(w + 1) * 512], bcp[:])

                # final P3 stripe: window 3
                for og in range(4):
                    for tcl in range(4):
                        emit_p3_tile(og, 12 + tcl)

    nc.compile()
    return nc


_PROGRAM = None


def _get_program():
    global _PROGRAM
    if _PROGRAM is None:
        _PROGRAM = build_program()
    return _PROGRAM


def make_in_maps(x, W_qkv, W_out):
    in_maps = []
    for c in range(8):
        b, t = c // 2, c % 2
        xt = np.ascontiguousarray(x[b].T).astype(BF)
        wq = np.ascontiguousarray(W_qkv[:, t * 1024:(t + 1) * 1024]).astype(BF)
        wk = W_qkv[:, D + t * 256: D + (t + 1) * 256]
        wv = W_qkv[:, D + 512 + t * 256: D + 512 + (t + 1) * 256]
        wkv = np.ascontiguousarray(np.concatenate([wk, wv], axis=1)).astype(BF)
        wo = np.ascontiguousarray(W_out[t * 1024:(t + 1) * 1024, :]).astype(BF)
        in_maps.append({"xt": xt, "wq": wq, "wkv": wkv, "wo": wo})
    return in_maps


def kernel(x, W_qkv, W_out):
    from concourse.bass_utils import run_bass_kernel_spmd
    nc = _get_program()
    in_maps = make_in_maps(np.asarray(x, dtype=np.float32),
                           np.asarray(W_qkv, dtype=np.float32),
                           np.asarray(W_out, dtype=np.float32))
    res = run_bass_kernel_spmd(nc, in_maps, list(range(8)), trace=False)
    out = np.empty((B, S, D), dtype=np.float32)
    for b in range(B):
        out[b] = (res.results[2 * b]["out"].astype(np.float32)
                  + res.results[2 * b + 1]["out"].astype(np.float32))
    return out
